# revision 53
# baseline (speedup 1.0000x reference)
"""Trainium2 Bass kernel for causal MHA (B=4, L=2048, D=1024, H=16), 8 cores.

Sharding: data-parallel over batch (4) x tensor-parallel over heads (2).
Each core handles one batch element and 8 heads.

v2 design (vs baseline):
  - bf16 operands everywhere (PSUM accumulation stays fp32); rel-err budget
    is 2e-2 and bf16 lands ~2e-3.
  - Everything SBUF-resident: x, Q/K, V (with fused ones-row for the softmax
    denominator), attention output, all weights. No DRAM bounce for V.
  - Score matmuls for the two heads of a pair issue back-to-back at base
    partitions 0/64 so the K=64 pairs run concurrently in different PE row
    groups.
  - exp() issued 1024 wide (st PSUM tiles span 2 banks) to amortize ACT
    per-instruction overhead; diagonal groups split to skip masked columns.
  - QKV projection matmul chains for pair p+1 are interleaved into pair p's
    attention emission (and out-projection chains into pair 3's) so the
    tensor queue always has ready work: keeps the PE HAM clock warm.
  - Softmax denominators: ones-row of V gives den in PSUM row 64; DVE
    reciprocal on a DMA-folded [8,128] tile; K=1 broadcast matmul spreads
    1/den across 64 partitions for the normalization multiplies.
"""

import collections
import contextlib

import numpy as np
import ml_dtypes

import concourse.bass as bass
import concourse.bacc as bacc
import concourse.mybir as mybir
import concourse.tile as tile

P = 128
HD = 64  # head dim

F32 = mybir.dt.float32
BF16 = mybir.dt.bfloat16


def build_mha_nc(L, D, HEADS):
    """Build the per-core Bass program. One batch element, HEADS heads."""
    DBLK = D // P          # contraction blocks for projections (8)
    KB = L // P            # key blocks (16)
    MC = L // 512          # token chunks for projections (4)
    PAIRS = HEADS // 2     # head pairs (4)
    EV = HEADS * HD        # v channels per core (512)
    EQK = 2 * HEADS * HD   # q+k rows per core (1024)
    ET = EQK // P          # qk tiles: per pair, one q-tile + one k-tile (8)
    NQ = L // 512          # query spans (4)
    scale = 1.0 / float(np.sqrt(HD))

    nc = bacc.Bacc("TRN2", target_bir_lowering=False, debug=False,
                   enable_asserts=False)

    xT = nc.dram_tensor("xT", [D, L], BF16, kind="ExternalInput").ap()
    wqkT = nc.dram_tensor("wqkT", [D, EQK], BF16, kind="ExternalInput").ap()
    wvT = nc.dram_tensor("wvT", [D, EV], BF16, kind="ExternalInput").ap()
    bqk = nc.dram_tensor("bqk", [P, ET], F32, kind="ExternalInput").ap()
    vb = nc.dram_tensor("vb", [P, EV], BF16, kind="ExternalInput").ap()
    woT = nc.dram_tensor("woT", [EV, D], BF16, kind="ExternalInput").ap()
    ob = nc.dram_tensor("ob", [P, D], F32, kind="ExternalInput").ap()
    tri = nc.dram_tensor("tri", [P, P], BF16, kind="ExternalInput").ap()
    onesd = nc.dram_tensor("onesd", [P, 512], BF16, kind="ExternalInput").ap()
    out = nc.dram_tensor("out", [L, D], F32, kind="ExternalOutput").ap()

    with tile.TileContext(nc) as tc:
        ctx = contextlib.ExitStack()
        with ctx:
            consts = ctx.enter_context(tc.tile_pool(name="consts", bufs=1))
            qk_pool = ctx.enter_context(tc.tile_pool(name="qk", bufs=5))
            ex_pool = ctx.enter_context(tc.tile_pool(name="ex", bufs=6))
            drow_pool = ctx.enter_context(tc.tile_pool(name="drow", bufs=3))
            tmp_pool = ctx.enter_context(tc.tile_pool(name="tmp", bufs=3))
            den_pool = ctx.enter_context(tc.tile_pool(name="den", bufs=3))
            recl_pool = ctx.enter_context(tc.tile_pool(name="recl", bufs=3))
            outst_pool = ctx.enter_context(tc.tile_pool(name="outst", bufs=4))
            st_ps = ctx.enter_context(
                tc.tile_pool(name="st_ps", bufs=2, space="PSUM"))
            av_ps = ctx.enter_context(
                tc.tile_pool(name="av_ps", bufs=2, space="PSUM"))
            mm_ps = ctx.enter_context(
                tc.tile_pool(name="mm_ps", bufs=2, space="PSUM"))

            # ---- SBUF-resident tensors ----
            tri_sb = consts.tile([P, P], BF16, name="tri_sb")
            ones_sb = consts.tile([P, 512], BF16, name="ones_sb")
            bqk_sb = consts.tile([P, ET], F32, name="bqk_sb")
            vb_sb = consts.tile([P, EV], BF16, name="vb_sb")
            ob_sb = consts.tile([P, D], F32, name="ob_sb")
            x_sb = consts.tile([P, DBLK, L], BF16, name="x_sb")
            wqk_sb = consts.tile([P, DBLK, EQK], BF16, name="wqk_sb")
            wv_sb = consts.tile([P, DBLK, EV], BF16, name="wv_sb")
            wo_sb = consts.tile([P, EV // P, D], BF16, name="wo_sb")
            v_all = consts.tile([P, KB, HEADS, HD + 1], BF16, name="v_all")
            attn_sb = consts.tile([P, PAIRS, L], BF16, name="attn_sb")
            # partial out-projection accumulator (pairs 0+1 contribution,
            # bias included) — lets half the out-proj run as filler during
            # pairs 1-2's attention
            ypart = consts.tile([P, KB, 2, 512], BF16, name="ypart")

            # small consts on the gpsimd queue; ones first (heater dep)
            nc.gpsimd.dma_start(out=ones_sb, in_=onesd)
            nc.gpsimd.dma_start(out=tri_sb, in_=tri)
            nc.gpsimd.dma_start(out=bqk_sb, in_=bqk)
            nc.gpsimd.dma_start(out=vb_sb, in_=vb)
            nc.gpsimd.dma_start(out=ob_sb, in_=ob)

            # big loads, chunked so first matmuls can start early; weights
            # on the scalar queue run parallel to x on the sync queue
            _wv_src = wvT.rearrange("(o p) e -> p o e", p=P)
            for o in range(0, DBLK, DBLK // 2):
                nc.scalar.dma_start(out=wv_sb[:, o:o + DBLK // 2, :],
                                    in_=_wv_src[:, o:o + DBLK // 2, :])
            _wqk_src = wqkT.rearrange("(o p) e -> p o e", p=P)
            _wo_src = woT.rearrange("(j p) f -> p j f", p=P)
            # all weight slices up front (pair 0 first) so attention-phase
            # filler units never stall on an in-flight weight DMA
            for pr_ in range(PAIRS):
                nc.scalar.dma_start(
                    out=wqk_sb[:, :, pr_ * 2 * P:(pr_ + 1) * 2 * P],
                    in_=_wqk_src[:, :, pr_ * 2 * P:(pr_ + 1) * 2 * P])
            for j in range(0, EV // P, 2):
                nc.scalar.dma_start(out=wo_sb[:, j:j + 2, :],
                                    in_=_wo_src[:, j:j + 2, :])
            _x_src = xT.rearrange("(o p) m -> p o m", p=P)
            for mc in range(MC):
                nc.sync.dma_start(
                    out=x_sb[:, :, mc * 512:(mc + 1) * 512],
                    in_=_x_src[:, :, mc * 512:(mc + 1) * 512])

            # ones column of V (softmax denominator trick)
            nc.vector.memset(v_all[:, :, :, HD:HD + 1], 1.0)

            # PE heater: junk matmuls during the startup DMA window so the
            # HAM clock gate is released (K=8/8) before real work arrives.
            # Results are never read.
            heat_ps = mm_ps.tile([P, 512], F32, name="heat_ps", tag="mm")
            for _ in range(28):
                nc.tensor.matmul(heat_ps, lhsT=ones_sb[:, 0:P], rhs=ones_sb,
                                 start=True, stop=True)

            qk_tiles = {}
            heat_n = [0]
            pin_after = [None]  # slot anchor: stops filler-unit hoisting

            def pin(mm):
                # anchor heaters to their slot (they have no natural deps
                # and would otherwise be hoisted early by the scheduler)
                if pin_after[0] is not None:
                    tile.add_dep_helper(mm.ins, pin_after[0].ins, sync=False,
                                        reason="pin heater to its slot")
                return mm

            def heater_unit(n=6):
                # junk matmuls: keeps the PE activity monitor busy through
                # slots with no real filler (prevents clock-gate collapse)
                heat_n[0] += 1
                hp = mm_ps.tile([P, 512], F32, name=f"heatu_{heat_n[0]}",
                                tag="mm")
                for k in range(n):
                    mm = nc.tensor.matmul(hp, lhsT=ones_sb[:, 0:P],
                                          rhs=ones_sb, start=True, stop=True)
                    if k == 0:
                        pin(mm)

            # ---------- work units (each: one PSUM accumulation chain) ----
            def v_unit(kb):
                ps = mm_ps.tile([P, 512], F32, name=f"vps_{kb}", tag="mm")
                for o in range(DBLK):
                    mm = nc.tensor.matmul(
                        ps,
                        lhsT=x_sb[:, o, kb * P:(kb + 1) * P],
                        rhs=wv_sb[:, o, :],
                        start=(o == 0), stop=(o == DBLK - 1))
                nc.vector.tensor_add(
                    out=v_all[:, kb, :, 0:HD],
                    in0=ps.rearrange("p (h c) -> p h c", c=HD),
                    in1=vb_sb.rearrange("p (h c) -> p h c", c=HD))

            def qk_unit(pr, t, mc):
                et = 2 * pr + t
                key = (pr, t)
                if key not in qk_tiles:
                    qk_tiles[key] = qk_pool.tile([P, L], BF16,
                                                 name=f"qk_{pr}_{t}", tag="qk")
                ps = mm_ps.tile([P, 512], F32, name=f"qkps_{et}_{mc}",
                                tag="mm")
                for o in range(DBLK):
                    mm = nc.tensor.matmul(
                        ps,
                        lhsT=wqk_sb[:, o, et * P:(et + 1) * P],
                        rhs=x_sb[:, o, mc * 512:(mc + 1) * 512],
                        start=(o == 0), stop=(o == DBLK - 1))
                nc.vector.tensor_scalar(
                    out=qk_tiles[key][:, mc * 512:(mc + 1) * 512],
                    in0=ps, scalar1=bqk_sb[:, et:et + 1], scalar2=None,
                    op0=mybir.AluOpType.add)

            def oproj_a(qt, fc):
                # pairs 0+1 contribution + bias -> ypart (bf16)
                f0 = fc * 512
                ps = mm_ps.tile([P, 512], F32, name=f"opa_{qt}_{fc}",
                                tag="mm")
                for j in (0, 1):
                    mm = nc.tensor.matmul(
                        ps,
                        lhsT=attn_sb[:, j, qt * P:(qt + 1) * P],
                        rhs=wo_sb[:, j, f0:f0 + 512],
                        start=(j == 0), stop=(j == 1))
                nc.vector.tensor_add(out=ypart[:, qt, fc, :], in0=ps,
                                     in1=ob_sb[:, f0:f0 + 512])

            def oproj_b2(qt, fc):
                # pair-2 contribution folded into ypart: depends only on
                # pair 2 -> ungated filler for pair 3's early groups
                f0 = fc * 512
                ps = mm_ps.tile([P, 512], F32, name=f"opb2_{qt}_{fc}",
                                tag="mm")
                nc.tensor.matmul(
                    ps,
                    lhsT=attn_sb[:, 2, qt * P:(qt + 1) * P],
                    rhs=wo_sb[:, 2, f0:f0 + 512],
                    start=True, stop=True)
                nc.vector.tensor_add(out=ypart[:, qt, fc, :], in0=ps,
                                     in1=ypart[:, qt, fc, :])

            def oproj_b3(qt, fc):
                # pair-3 contribution + ypart -> out (f32)
                f0 = fc * 512
                ps = mm_ps.tile([P, 512], F32, name=f"opb3_{qt}_{fc}",
                                tag="mm")
                nc.tensor.matmul(
                    ps,
                    lhsT=attn_sb[:, 3, qt * P:(qt + 1) * P],
                    rhs=wo_sb[:, 3, f0:f0 + 512],
                    start=True, stop=True)
                ot = outst_pool.tile([P, 512], F32, name=f"ot_{qt}_{fc}",
                                     tag="outst")
                nc.vector.tensor_add(out=ot, in0=ps, in1=ypart[:, qt, fc, :])
                nc.sync.dma_start(
                    out=out[qt * P:(qt + 1) * P, f0:f0 + 512], in_=ot)

            # ---------- attention for one pair, with filler interleave ----
            # Deferred softmax-normalization stages. The reciprocal/broadcast
            # chain of token span s is emitted one span LATER (between that
            # span's score groups) so it never sits ahead of ready matmuls
            # in the tensor queue. pending_norm carries stages across q4 and
            # pair boundaries.
            pending_norm = []

            def attention_pair(pr, fill_plan):
                """fill_plan: {sequential group idx -> [callables]} of
                projection / out-proj units, emitted between a group's exps
                and its AV matmuls so the PE has independent work while the
                ACT engine runs the exps."""
                q_tile = qk_tiles[(pr, 0)]
                k_tile = qk_tiles[(pr, 1)]
                gidx = 0
                for q4 in range(NQ):
                    q0 = q4 * 512
                    nkb = 4 * (q4 + 1)
                    G = nkb // 2
                    avs = []
                    for hh in (0, 1):
                        av = av_ps.tile([HD + 1, 512], F32,
                                        name=f"av_{pr}_{hh}_{q4}", tag="av")
                        avs.append(av)
                    for g in range(G):
                        kbs = (2 * g, 2 * g + 1)
                        # kb-major tiles: each holds BOTH heads for one kb
                        # ([h0 | h1] across 2 banks) so exp fires after just
                        # the two concurrent score matmuls of that kb
                        sts = []
                        exs = []
                        for i, kb in enumerate(kbs):
                            s0 = max(0, kb * P - q0)
                            st = st_ps.tile([P, 1024], F32,
                                            name=f"st_{pr}_{q4}_{g}_{i}",
                                            tag="st")
                            # h0/h1 back-to-back: K=64 pairs run in
                            # different PE row groups concurrently
                            pair_mms = []
                            for hh in (0, 1):
                                rows = slice(hh * HD, hh * HD + HD)
                                pair_mms.append(nc.tensor.matmul(
                                    st[:, hh * 512 + s0:hh * 512 + 512],
                                    lhsT=k_tile[rows, kb * P:(kb + 1) * P],
                                    rhs=q_tile[rows, q0 + s0:q0 + 512],
                                    start=True, stop=True))
                            tile.add_dep_helper(
                                pair_mms[1].ins, pair_mms[0].ins, sync=False,
                                reason="keep score pair adjacent")
                            pin_after[0] = pair_mms[0]
                            sts.append(st)
                            # exp: one wide call unless the diagonal trim
                            # makes two narrow calls cheaper
                            ex = ex_pool.tile([P, 1024], BF16,
                                              name=f"ex_{pr}_{q4}_{g}_{i}",
                                              tag="ex")
                            if s0 <= P:
                                nc.scalar.activation(
                                    out=ex[:, s0:1024], in_=st[:, s0:1024],
                                    func=mybir.ActivationFunctionType.Exp,
                                    scale=scale)
                            else:
                                nc.scalar.activation(
                                    out=ex[:, s0:512], in_=st[:, s0:512],
                                    func=mybir.ActivationFunctionType.Exp,
                                    scale=scale)
                                nc.scalar.activation(
                                    out=ex[:, 512 + s0:1024],
                                    in_=st[:, 512 + s0:1024],
                                    func=mybir.ActivationFunctionType.Exp,
                                    scale=scale)
                            # causal mask on the diagonal block (gpsimd:
                            # off the busier vector queue)
                            d0 = kb * P - q0
                            if d0 >= 0:
                                for hh in (0, 1):
                                    c = hh * 512 + d0
                                    nc.gpsimd.tensor_mul(
                                        out=ex[:, c:c + P],
                                        in0=ex[:, c:c + P], in1=tri_sb)
                            exs.append(ex)
                        # deferred norms + filler land here: between the
                        # exps and their AV consumers, so the PE crosses the
                        # ACT latency on independent work
                        if g <= 1 and pending_norm:
                            pending_norm.pop(0)()
                        for fn in fill_plan.get(gidx, ()):
                            fn()
                        gidx += 1
                        # AV accumulation (kb-major: kb0's avs only need
                        # kb0's exp)
                        for i, kb in enumerate(kbs):
                            s0 = max(0, kb * P - q0)
                            for hh in (0, 1):
                                h = 2 * pr + hh
                                nc.tensor.matmul(
                                    avs[hh][:, s0:512],
                                    lhsT=v_all[:, kb, h, :],
                                    rhs=exs[i][:, hh * 512 + s0:
                                               hh * 512 + 512],
                                    start=(kb == 0), stop=(kb == nkb - 1))

                    # ---- q4 epilogue: evacuate now, normalize deferred ----
                    dr = drow_pool.tile([P, 512], BF16, name=f"dr_{pr}_{q4}",
                                        tag="drow")
                    tmpt = tmp_pool.tile([P, 512], BF16, name=f"tp_{pr}_{q4}",
                                         tag="tmp")
                    den = den_pool.tile([8, P], BF16, name=f"den_{pr}_{q4}",
                                        tag="den")
                    recl = recl_pool.tile([P, 2, 512], BF16,
                                          name=f"recl_{pr}_{q4}", tag="recl")
                    # h0: raw attn to attn_sb, den row to dr
                    nc.vector.tensor_copy(
                        out=attn_sb[0:HD, pr, q0:q0 + 512], in_=avs[0][0:HD, :])
                    nc.vector.tensor_copy(out=dr[HD:HD + 1, :],
                                          in_=avs[0][HD:HD + 1, :])
                    # h1: raw attn + den row to tmp
                    nc.vector.tensor_copy(out=tmpt[0:HD + 1, :], in_=avs[1])
                    # fold den rows [1,512] -> [4,128] for a cheap reciprocal
                    nc.gpsimd.dma_start(out=den[0:4, :], in_=dr[HD:HD + 1, :])
                    nc.gpsimd.dma_start(out=den[4:8, :],
                                        in_=tmpt[HD:HD + 1, :])

                    def norm_a(den=den, recl=recl):
                        with nc.allow_low_precision(
                                reason="bf16 rounding of softmax denom"):
                            nc.vector.reciprocal(out=den, in_=den)
                        # unfold to partition 64 (legal matmul base partition)
                        nc.gpsimd.dma_start(out=recl[HD:HD + 1, 0, :],
                                            in_=den[0:4, :])
                        nc.gpsimd.dma_start(out=recl[HD:HD + 1, 1, :],
                                            in_=den[4:8, :])

                    def norm_b(pr=pr, q0=q0, recl=recl, tmpt=tmpt):
                        # broadcast 1/den across 64 partitions via K=1 matmul
                        bpst = st_ps.tile([P, 1024], F32,
                                          name=f"bps_{pr}_{q0}", tag="st")
                        for hh in (0, 1):
                            nc.tensor.matmul(
                                bpst[0:HD, hh * 512:hh * 512 + 512],
                                lhsT=ones_sb[HD:HD + 1, 0:HD],
                                rhs=recl[HD:HD + 1, hh, :],
                                start=True, stop=True)
                        sl = attn_sb[0:HD, pr, q0:q0 + 512]
                        nc.vector.tensor_mul(out=sl, in0=sl,
                                             in1=bpst[0:HD, 0:512])
                        nc.vector.tensor_mul(out=tmpt[0:HD, :],
                                             in0=tmpt[0:HD, :],
                                             in1=bpst[0:HD, 512:1024])
                        nc.gpsimd.dma_start(
                            out=attn_sb[HD:P, pr, q0:q0 + 512],
                            in_=tmpt[0:HD, :])

                    pending_norm.append(norm_a)
                    pending_norm.append(norm_b)

            # ---------- emission schedule ----------
            # Minimal prologue: just what pair-0 span 0 needs. The rest of
            # the V/QK0 projections ride inside pair 0's attention as
            # deadline-constrained filler — this keeps the attention region
            # PE-bound (ACT would otherwise dominate it and the PE clock
            # gate would oscillate).
            for kb in range(4):
                v_unit(kb)
            qk_unit(0, 0, 0)
            qk_unit(0, 1, 0)

            # Pairs: each pair's attention (20 groups, indices 0..19 —
            # q4=0: 0-1, q4=1: 2-5, q4=2: 6-11, q4=3: 12-19) carries filler
            # units, each tagged with the earliest legal group.
            def build_plan(jobs, nslots=20, heat_from=20):
                """jobs: [(min_slot, max_slot, callable)] -> {slot: [fns]},
                greedily balancing load within each job's legal window.
                Empty slots get a heater; slots >= heat_from get one extra
                (late-pair insurance against clock-gate collapse)."""
                plan = collections.defaultdict(list)
                for ms, mx, fn in sorted(jobs, key=lambda j: (j[1], j[0])):
                    slot = min(range(ms, mx + 1),
                               key=lambda s_: (len(plan[s_]), s_))
                    plan[slot].append(fn)
                return plan

            for pr in range(PAIRS):
                jobs = []
                if pr == 0:
                    # rest of the V projection and own QK, due before the
                    # span that consumes them (q4=s starts at slot
                    # {1:2, 2:6, 3:12})
                    win = {1: (0, 1), 2: (2, 5), 3: (6, 11)}
                    for blk in (1, 2, 3):
                        lo, hi = win[blk]
                        jobs += [(lo, hi, lambda kb=kb: v_unit(kb))
                                 for kb in range(4 * blk, 4 * blk + 4)]
                        jobs += [(lo, hi,
                                  lambda t=t, blk=blk: qk_unit(0, t, blk))
                                 for t in (0, 1)]
                if pr + 1 < PAIRS:
                    # next pair's qk units (pair 2 keeps mc3 back: pair 3
                    # fills its own first groups with them)
                    jobs += [
                        (0, 19, lambda t=t, mc=mc: qk_unit(pr + 1, t, mc))
                        for mc in range(MC if pr < 2 else MC - 1)
                        for t in (0, 1)]
                else:
                    jobs += [(0, 0, lambda: qk_unit(3, 0, 3)),
                             (1, 1, lambda: qk_unit(3, 1, 3))]
                if pr == 1:
                    # out-proj part A (pairs 0+1) for spans this pair has
                    # normalized: span s legal once norm_b(p1, s) popped
                    # (q4=s+1, g>=2)
                    span_min = {0: 4, 1: 8}
                    for s in (0, 1):
                        jobs += [
                            (span_min[s], 19,
                             lambda qt=qt, fc=fc: oproj_a(qt, fc))
                            for qt in range(4 * s, 4 * s + 4)
                            for fc in (0, 1)]
                elif pr == 2:
                    # part A for spans 2-3; span 3 must wait for pair 1's
                    # deferred span-3 norms (popped at slots 0-1)
                    jobs += [
                        (0 if qt < 12 else 2, 19,
                         lambda qt=qt, fc=fc: oproj_a(qt, fc))
                        for qt in range(8, KB) for fc in (0, 1)]
                elif pr == 3:
                    # j2 half of part B: ungated (pair 2 done) except span 3
                    # whose pair-2 norm pops at slots 0-1. Each b2 window
                    # closes before its span's b3 window opens (b3 reads the
                    # ypart that b2 writes).
                    b2win = {0: (0, 3), 1: (0, 7), 2: (0, 13), 3: (2, 16)}
                    for s in range(4):
                        lo, hi = b2win[s]
                        jobs += [
                            (lo, hi, lambda qt=qt, fc=fc: oproj_b2(qt, fc))
                            for qt in range(4 * s, 4 * s + 4)
                            for fc in (0, 1)]
                    # j3 half for spans this pair has finished
                    span_min = {0: 4, 1: 8, 2: 14}
                    for s in (0, 1, 2):
                        jobs += [
                            (span_min[s], 19,
                             lambda qt=qt, fc=fc: oproj_b3(qt, fc))
                            for qt in range(4 * s, 4 * s + 4)
                            for fc in (0, 1)]
                attention_pair(pr, build_plan(jobs))

            # flush deferred normalizations, then the last token span's
            # out-projection part B
            while pending_norm:
                pending_norm.pop(0)()
            for qt in range(12, KB):
                for fc in (0, 1):
                    oproj_b3(qt, fc)

    nc.compile()
    return nc


def make_core_inputs(x, Wqkv_w, Wqkv_b, out_w, out_b, H, n_tp):
    """Host-side shard + layout prep. Returns list of in_maps (one per core).
    Core c handles batch c // n_tp, head group c % n_tp."""
    B, L, D = x.shape
    hpg = H // n_tp            # heads per core
    EV = hpg * HD
    ET = 2 * hpg * HD // P
    bf = ml_dtypes.bfloat16
    tri = np.triu(np.ones((P, P), dtype=np.float32))  # [k, q]: 1 if q >= k
    in_maps = []
    for c in range(B * n_tp):
        b, g = c // n_tp, c % n_tp
        # qk row order: per pair p -> q(2p), q(2p+1), k(2p), k(2p+1)
        qk_rows = []
        for p_ in range(hpg // 2):
            for h in (2 * p_, 2 * p_ + 1):
                qk_rows.extend(range(g * hpg * HD + h * HD,
                                     g * hpg * HD + h * HD + HD))
            for h in (2 * p_, 2 * p_ + 1):
                qk_rows.extend(range(D + g * hpg * HD + h * HD,
                                     D + g * hpg * HD + h * HD + HD))
        qk_rows = np.array(qk_rows)
        v_rows = np.arange(2 * D + g * EV, 2 * D + (g + 1) * EV)
        in_maps.append({
            "xT": np.ascontiguousarray(x[b].T).astype(bf),
            "wqkT": np.ascontiguousarray(Wqkv_w[qk_rows].T).astype(bf),
            "wvT": np.ascontiguousarray(Wqkv_w[v_rows].T).astype(bf),
            "bqk": np.ascontiguousarray(
                Wqkv_b[qk_rows].reshape(ET, P).T).astype(np.float32),
            "vb": np.tile(Wqkv_b[v_rows], (P, 1)).astype(bf),
            "woT": np.ascontiguousarray(
                out_w[:, g * EV:(g + 1) * EV].T).astype(bf),
            "ob": (np.tile(out_b, (P, 1)).astype(np.float32) if g == 0
                   else np.zeros((P, D), np.float32)),
            "tri": tri.astype(bf),
            "onesd": np.ones((P, 512), np.float32).astype(bf),
        })
    return in_maps


_NC_CACHE = {}
LAST_RESULTS = None


def kernel(x, Wqkv_w, Wqkv_b, out_w, out_b):
    global LAST_RESULTS
    x = np.asarray(x, dtype=np.float32)
    Wqkv_w = np.asarray(Wqkv_w, dtype=np.float32)
    Wqkv_b = np.asarray(Wqkv_b, dtype=np.float32)
    out_w = np.asarray(out_w, dtype=np.float32)
    out_b = np.asarray(out_b, dtype=np.float32)

    B, L, D = x.shape
    H = 16
    n_tp = 2
    hpg = H // n_tp

    key = (L, D, hpg)
    if key not in _NC_CACHE:
        _NC_CACHE[key] = build_mha_nc(L, D, hpg)
    nc = _NC_CACHE[key]

    in_maps = make_core_inputs(x, Wqkv_w, Wqkv_b, out_w, out_b, H, n_tp)

    from concourse.bass_utils import run_bass_kernel_spmd
    res = run_bass_kernel_spmd(nc, in_maps, core_ids=list(range(len(in_maps))))
    LAST_RESULTS = res

    out = np.empty((B, L, D), dtype=np.float32)
    for b in range(B):
        out[b] = res.results[n_tp * b]["out"]
        for g in range(1, n_tp):
            out[b] += res.results[n_tp * b + g]["out"]
    return out


if __name__ == "__main__":
    nc = build_mha_nc(2048, 1024, 8)
    print("built OK")


# revision 55
# speedup vs baseline: 1.0300x; 1.0300x over previous
"""Trainium2 Bass kernel for causal MHA (B=4, L=2048, D=1024, H=16), 8 cores.

Sharding: data-parallel over batch (4) x tensor-parallel over heads (2).
Each core handles one batch element and 8 heads.

v2 design (vs baseline):
  - bf16 operands everywhere (PSUM accumulation stays fp32); rel-err budget
    is 2e-2 and bf16 lands ~2e-3.
  - Everything SBUF-resident: x, Q/K, V (with fused ones-row for the softmax
    denominator), attention output, all weights. No DRAM bounce for V.
  - Score matmuls for the two heads of a pair issue back-to-back at base
    partitions 0/64 so the K=64 pairs run concurrently in different PE row
    groups.
  - exp() issued 1024 wide (st PSUM tiles span 2 banks) to amortize ACT
    per-instruction overhead; diagonal groups split to skip masked columns.
  - QKV projection matmul chains for pair p+1 are interleaved into pair p's
    attention emission (and out-projection chains into pair 3's) so the
    tensor queue always has ready work: keeps the PE HAM clock warm.
  - Softmax denominators: ones-row of V gives den in PSUM row 64; DVE
    reciprocal on a DMA-folded [8,128] tile; K=1 broadcast matmul spreads
    1/den across 64 partitions for the normalization multiplies.
"""

import collections
import contextlib

import numpy as np
import ml_dtypes

import concourse.bass as bass
import concourse.bacc as bacc
import concourse.mybir as mybir
import concourse.tile as tile

P = 128
HD = 64  # head dim

F32 = mybir.dt.float32
BF16 = mybir.dt.bfloat16


def build_mha_nc(L, D, HEADS):
    """Build the per-core Bass program. One batch element, HEADS heads."""
    DBLK = D // P          # contraction blocks for projections (8)
    KB = L // P            # key blocks (16)
    MC = L // 512          # token chunks for projections (4)
    PAIRS = HEADS // 2     # head pairs (4)
    EV = HEADS * HD        # v channels per core (512)
    EQK = 2 * HEADS * HD   # q+k rows per core (1024)
    ET = EQK // P          # qk tiles: per pair, one q-tile + one k-tile (8)
    NQ = L // 512          # query spans (4)
    scale = 1.0 / float(np.sqrt(HD))

    nc = bacc.Bacc("TRN2", target_bir_lowering=False, debug=False,
                   enable_asserts=False)

    xT = nc.dram_tensor("xT", [D, L], BF16, kind="ExternalInput").ap()
    wqkT = nc.dram_tensor("wqkT", [D, EQK], BF16, kind="ExternalInput").ap()
    wvT = nc.dram_tensor("wvT", [D, EV], BF16, kind="ExternalInput").ap()
    bqk = nc.dram_tensor("bqk", [P, ET], F32, kind="ExternalInput").ap()
    vb = nc.dram_tensor("vb", [P, EV], BF16, kind="ExternalInput").ap()
    woT = nc.dram_tensor("woT", [EV, D], BF16, kind="ExternalInput").ap()
    ob = nc.dram_tensor("ob", [P, D], F32, kind="ExternalInput").ap()
    tri = nc.dram_tensor("tri", [P, P], BF16, kind="ExternalInput").ap()
    onesd = nc.dram_tensor("onesd", [P, 512], BF16, kind="ExternalInput").ap()
    out = nc.dram_tensor("out", [L, D], F32, kind="ExternalOutput").ap()

    with tile.TileContext(nc) as tc:
        ctx = contextlib.ExitStack()
        with ctx:
            consts = ctx.enter_context(tc.tile_pool(name="consts", bufs=1))
            qk_pool = ctx.enter_context(tc.tile_pool(name="qk", bufs=6))
            ex_pool = ctx.enter_context(tc.tile_pool(name="ex", bufs=10))
            drow_pool = ctx.enter_context(tc.tile_pool(name="drow", bufs=3))
            tmp_pool = ctx.enter_context(tc.tile_pool(name="tmp", bufs=3))
            den_pool = ctx.enter_context(tc.tile_pool(name="den", bufs=3))
            recl_pool = ctx.enter_context(tc.tile_pool(name="recl", bufs=3))
            outst_pool = ctx.enter_context(tc.tile_pool(name="outst", bufs=8))
            st_ps = ctx.enter_context(
                tc.tile_pool(name="st_ps", bufs=2, space="PSUM"))
            av_ps = ctx.enter_context(
                tc.tile_pool(name="av_ps", bufs=2, space="PSUM"))
            mm_ps = ctx.enter_context(
                tc.tile_pool(name="mm_ps", bufs=2, space="PSUM"))

            # ---- SBUF-resident tensors ----
            tri_sb = consts.tile([P, P], BF16, name="tri_sb")
            ones_sb = consts.tile([P, 512], BF16, name="ones_sb")
            bqk_sb = consts.tile([P, ET], F32, name="bqk_sb")
            vb_sb = consts.tile([P, EV], BF16, name="vb_sb")
            ob_sb = consts.tile([P, D], F32, name="ob_sb")
            x_sb = consts.tile([P, DBLK, L], BF16, name="x_sb")
            wqk_sb = consts.tile([P, DBLK, EQK], BF16, name="wqk_sb")
            wv_sb = consts.tile([P, DBLK, EV], BF16, name="wv_sb")
            wo_sb = consts.tile([P, EV // P, D], BF16, name="wo_sb")
            v_all = consts.tile([P, KB, HEADS, HD + 1], BF16, name="v_all")
            attn_sb = consts.tile([P, PAIRS, L], BF16, name="attn_sb")
            # partial out-projection accumulator (pairs 0+1 contribution,
            # bias included) — lets half the out-proj run as filler during
            # pairs 1-2's attention
            ypart = consts.tile([P, KB, 2, 512], BF16, name="ypart")

            # small consts on the gpsimd queue; ones first (heater dep)
            nc.gpsimd.dma_start(out=ones_sb, in_=onesd)
            nc.gpsimd.dma_start(out=tri_sb, in_=tri)
            nc.gpsimd.dma_start(out=bqk_sb, in_=bqk)
            nc.gpsimd.dma_start(out=vb_sb, in_=vb)
            nc.gpsimd.dma_start(out=ob_sb, in_=ob)

            # big loads, chunked so first matmuls can start early; weights
            # on the scalar queue run parallel to x on the sync queue
            _wv_src = wvT.rearrange("(o p) e -> p o e", p=P)
            for o in range(0, DBLK, DBLK // 2):
                nc.scalar.dma_start(out=wv_sb[:, o:o + DBLK // 2, :],
                                    in_=_wv_src[:, o:o + DBLK // 2, :])
            _wqk_src = wqkT.rearrange("(o p) e -> p o e", p=P)
            _wo_src = woT.rearrange("(j p) f -> p j f", p=P)
            # all weight slices up front (pair 0 first) so attention-phase
            # filler units never stall on an in-flight weight DMA
            for pr_ in range(PAIRS):
                nc.scalar.dma_start(
                    out=wqk_sb[:, :, pr_ * 2 * P:(pr_ + 1) * 2 * P],
                    in_=_wqk_src[:, :, pr_ * 2 * P:(pr_ + 1) * 2 * P])
            for j in range(0, EV // P, 2):
                nc.scalar.dma_start(out=wo_sb[:, j:j + 2, :],
                                    in_=_wo_src[:, j:j + 2, :])
            _x_src = xT.rearrange("(o p) m -> p o m", p=P)
            for mc in range(MC):
                nc.sync.dma_start(
                    out=x_sb[:, :, mc * 512:(mc + 1) * 512],
                    in_=_x_src[:, :, mc * 512:(mc + 1) * 512])

            # ones column of V (softmax denominator trick)
            nc.vector.memset(v_all[:, :, :, HD:HD + 1], 1.0)

            # PE heater: junk matmuls during the startup DMA window so the
            # HAM clock gate is released (K=8/8) before real work arrives.
            # Results are never read.
            heat_ps = mm_ps.tile([P, 512], F32, name="heat_ps", tag="mm")
            for _ in range(36):
                nc.tensor.matmul(heat_ps, lhsT=ones_sb[:, 0:P], rhs=ones_sb,
                                 start=True, stop=True)

            qk_tiles = {}
            heat_n = [0]
            pin_after = [None]  # slot anchor: stops filler-unit hoisting

            def pin(mm):
                # anchor heaters to their slot (they have no natural deps
                # and would otherwise be hoisted early by the scheduler)
                if pin_after[0] is not None:
                    tile.add_dep_helper(mm.ins, pin_after[0].ins, sync=False,
                                        reason="pin heater to its slot")
                return mm

            def heater_unit(n=6):
                # junk matmuls: keeps the PE activity monitor busy through
                # slots with no real filler (prevents clock-gate collapse)
                heat_n[0] += 1
                hp = mm_ps.tile([P, 512], F32, name=f"heatu_{heat_n[0]}",
                                tag="mm")
                for k in range(n):
                    mm = nc.tensor.matmul(hp, lhsT=ones_sb[:, 0:P],
                                          rhs=ones_sb, start=True, stop=True)
                    if k == 0:
                        pin(mm)

            # ---------- work units (each: one PSUM accumulation chain) ----
            def v_unit(kb):
                ps = mm_ps.tile([P, 512], F32, name=f"vps_{kb}", tag="mm")
                for o in range(DBLK):
                    mm = nc.tensor.matmul(
                        ps,
                        lhsT=x_sb[:, o, kb * P:(kb + 1) * P],
                        rhs=wv_sb[:, o, :],
                        start=(o == 0), stop=(o == DBLK - 1))
                nc.vector.tensor_add(
                    out=v_all[:, kb, :, 0:HD],
                    in0=ps.rearrange("p (h c) -> p h c", c=HD),
                    in1=vb_sb.rearrange("p (h c) -> p h c", c=HD))

            def qk_unit(pr, t, mc):
                et = 2 * pr + t
                key = (pr, t)
                if key not in qk_tiles:
                    qk_tiles[key] = qk_pool.tile([P, L], BF16,
                                                 name=f"qk_{pr}_{t}", tag="qk")
                ps = mm_ps.tile([P, 512], F32, name=f"qkps_{et}_{mc}",
                                tag="mm")
                for o in range(DBLK):
                    mm = nc.tensor.matmul(
                        ps,
                        lhsT=wqk_sb[:, o, et * P:(et + 1) * P],
                        rhs=x_sb[:, o, mc * 512:(mc + 1) * 512],
                        start=(o == 0), stop=(o == DBLK - 1))
                nc.vector.tensor_scalar(
                    out=qk_tiles[key][:, mc * 512:(mc + 1) * 512],
                    in0=ps, scalar1=bqk_sb[:, et:et + 1], scalar2=None,
                    op0=mybir.AluOpType.add)

            def oproj_a(qt, fc):
                # pairs 0+1 contribution + bias -> ypart (bf16)
                f0 = fc * 512
                ps = mm_ps.tile([P, 512], F32, name=f"opa_{qt}_{fc}",
                                tag="mm")
                for j in (0, 1):
                    mm = nc.tensor.matmul(
                        ps,
                        lhsT=attn_sb[:, j, qt * P:(qt + 1) * P],
                        rhs=wo_sb[:, j, f0:f0 + 512],
                        start=(j == 0), stop=(j == 1))
                nc.vector.tensor_add(out=ypart[:, qt, fc, :], in0=ps,
                                     in1=ob_sb[:, f0:f0 + 512])

            def oproj_b(qt, fc):
                # pairs 2+3 contribution + ypart -> out (f32)
                f0 = fc * 512
                ps = mm_ps.tile([P, 512], F32, name=f"opb_{qt}_{fc}",
                                tag="mm")
                for j in (2, 3):
                    mm = nc.tensor.matmul(
                        ps,
                        lhsT=attn_sb[:, j, qt * P:(qt + 1) * P],
                        rhs=wo_sb[:, j, f0:f0 + 512],
                        start=(j == 2), stop=(j == 3))
                ot = outst_pool.tile([P, 512], F32, name=f"ot_{qt}_{fc}",
                                     tag="outst")
                nc.vector.tensor_add(out=ot, in0=ps, in1=ypart[:, qt, fc, :])
                nc.sync.dma_start(
                    out=out[qt * P:(qt + 1) * P, f0:f0 + 512], in_=ot)

            # ---------- attention for one pair, with filler interleave ----
            # Deferred softmax-normalization stages. The reciprocal/broadcast
            # chain of token span s is emitted one span LATER (between that
            # span's score groups) so it never sits ahead of ready matmuls
            # in the tensor queue. pending_norm carries stages across q4 and
            # pair boundaries.
            pending_norm = []

            def attention_pair(pr, fill_plan):
                """fill_plan: {sequential group idx -> [callables]} of
                projection / out-proj units, emitted between a group's exps
                and its AV matmuls so the PE has independent work while the
                ACT engine runs the exps."""
                q_tile = qk_tiles[(pr, 0)]
                k_tile = qk_tiles[(pr, 1)]
                gidx = 0
                for q4 in range(NQ):
                    q0 = q4 * 512
                    nkb = 4 * (q4 + 1)
                    G = nkb // 2
                    avs = []
                    for hh in (0, 1):
                        av = av_ps.tile([HD + 1, 512], F32,
                                        name=f"av_{pr}_{hh}_{q4}", tag="av")
                        avs.append(av)
                    for g in range(G):
                        kbs = (2 * g, 2 * g + 1)
                        # kb-major tiles: each holds BOTH heads for one kb
                        # ([h0 | h1] across 2 banks) so exp fires after just
                        # the two concurrent score matmuls of that kb
                        sts = []
                        exs = []
                        for i, kb in enumerate(kbs):
                            s0 = max(0, kb * P - q0)
                            st = st_ps.tile([P, 1024], F32,
                                            name=f"st_{pr}_{q4}_{g}_{i}",
                                            tag="st")
                            # h0/h1 back-to-back: K=64 pairs run in
                            # different PE row groups concurrently
                            pair_mms = []
                            for hh in (0, 1):
                                rows = slice(hh * HD, hh * HD + HD)
                                pair_mms.append(nc.tensor.matmul(
                                    st[:, hh * 512 + s0:hh * 512 + 512],
                                    lhsT=k_tile[rows, kb * P:(kb + 1) * P],
                                    rhs=q_tile[rows, q0 + s0:q0 + 512],
                                    start=True, stop=True))
                            tile.add_dep_helper(
                                pair_mms[1].ins, pair_mms[0].ins, sync=False,
                                reason="keep score pair adjacent")
                            pin_after[0] = pair_mms[0]
                            sts.append(st)
                            # exp: one wide call unless the diagonal trim
                            # makes two narrow calls cheaper
                            ex = ex_pool.tile([P, 1024], BF16,
                                              name=f"ex_{pr}_{q4}_{g}_{i}",
                                              tag="ex")
                            if s0 <= 2 * P:
                                nc.scalar.activation(
                                    out=ex[:, s0:1024], in_=st[:, s0:1024],
                                    func=mybir.ActivationFunctionType.Exp,
                                    scale=scale)
                            else:
                                nc.scalar.activation(
                                    out=ex[:, s0:512], in_=st[:, s0:512],
                                    func=mybir.ActivationFunctionType.Exp,
                                    scale=scale)
                                nc.scalar.activation(
                                    out=ex[:, 512 + s0:1024],
                                    in_=st[:, 512 + s0:1024],
                                    func=mybir.ActivationFunctionType.Exp,
                                    scale=scale)
                            # causal mask on the diagonal block (gpsimd:
                            # off the busier vector queue)
                            d0 = kb * P - q0
                            if d0 >= 0:
                                for hh in (0, 1):
                                    c = hh * 512 + d0
                                    nc.gpsimd.tensor_mul(
                                        out=ex[:, c:c + P],
                                        in0=ex[:, c:c + P], in1=tri_sb)
                            exs.append(ex)
                        # deferred norms + filler land here: between the
                        # exps and their AV consumers, so the PE crosses the
                        # ACT latency on independent work
                        if g <= 1 and pending_norm:
                            pending_norm.pop(0)()
                        for fn in fill_plan.get(gidx, ()):
                            fn()
                        gidx += 1
                        # AV accumulation (kb-major: kb0's avs only need
                        # kb0's exp)
                        for i, kb in enumerate(kbs):
                            s0 = max(0, kb * P - q0)
                            for hh in (0, 1):
                                h = 2 * pr + hh
                                nc.tensor.matmul(
                                    avs[hh][:, s0:512],
                                    lhsT=v_all[:, kb, h, :],
                                    rhs=exs[i][:, hh * 512 + s0:
                                               hh * 512 + 512],
                                    start=(kb == 0), stop=(kb == nkb - 1))

                    # ---- q4 epilogue: evacuate now, normalize deferred ----
                    dr = drow_pool.tile([P, 512], BF16, name=f"dr_{pr}_{q4}",
                                        tag="drow")
                    tmpt = tmp_pool.tile([P, 512], BF16, name=f"tp_{pr}_{q4}",
                                         tag="tmp")
                    den = den_pool.tile([8, P], BF16, name=f"den_{pr}_{q4}",
                                        tag="den")
                    recl = recl_pool.tile([P, 2, 512], BF16,
                                          name=f"recl_{pr}_{q4}", tag="recl")
                    # h0: raw attn to attn_sb, den row to dr
                    nc.vector.tensor_copy(
                        out=attn_sb[0:HD, pr, q0:q0 + 512], in_=avs[0][0:HD, :])
                    nc.vector.tensor_copy(out=dr[HD:HD + 1, :],
                                          in_=avs[0][HD:HD + 1, :])
                    # h1: raw attn + den row to tmp
                    nc.vector.tensor_copy(out=tmpt[0:HD + 1, :], in_=avs[1])
                    # fold den rows [1,512] -> [4,128] for a cheap reciprocal
                    nc.gpsimd.dma_start(out=den[0:4, :], in_=dr[HD:HD + 1, :])
                    nc.gpsimd.dma_start(out=den[4:8, :],
                                        in_=tmpt[HD:HD + 1, :])

                    def norm_a(den=den, recl=recl):
                        with nc.allow_low_precision(
                                reason="bf16 rounding of softmax denom"):
                            nc.vector.reciprocal(out=den, in_=den)
                        # unfold to partition 64 (legal matmul base partition)
                        nc.gpsimd.dma_start(out=recl[HD:HD + 1, 0, :],
                                            in_=den[0:4, :])
                        nc.gpsimd.dma_start(out=recl[HD:HD + 1, 1, :],
                                            in_=den[4:8, :])

                    def norm_b(pr=pr, q0=q0, recl=recl, tmpt=tmpt):
                        # broadcast 1/den across 64 partitions via K=1 matmul
                        bpst = st_ps.tile([P, 1024], F32,
                                          name=f"bps_{pr}_{q0}", tag="st")
                        for hh in (0, 1):
                            nc.tensor.matmul(
                                bpst[0:HD, hh * 512:hh * 512 + 512],
                                lhsT=ones_sb[HD:HD + 1, 0:HD],
                                rhs=recl[HD:HD + 1, hh, :],
                                start=True, stop=True)
                        sl = attn_sb[0:HD, pr, q0:q0 + 512]
                        nc.vector.tensor_mul(out=sl, in0=sl,
                                             in1=bpst[0:HD, 0:512])
                        nc.vector.tensor_mul(out=tmpt[0:HD, :],
                                             in0=tmpt[0:HD, :],
                                             in1=bpst[0:HD, 512:1024])
                        nc.gpsimd.dma_start(
                            out=attn_sb[HD:P, pr, q0:q0 + 512],
                            in_=tmpt[0:HD, :])

                    pending_norm.append(norm_a)
                    pending_norm.append(norm_b)

            # ---------- emission schedule ----------
            # Minimal prologue: just what pair-0 span 0 needs. The rest of
            # the V/QK0 projections ride inside pair 0's attention as
            # deadline-constrained filler — this keeps the attention region
            # PE-bound (ACT would otherwise dominate it and the PE clock
            # gate would oscillate).
            for kb in range(4):
                v_unit(kb)
            qk_unit(0, 0, 0)
            qk_unit(0, 1, 0)

            # Pairs: each pair's attention (20 groups, indices 0..19 —
            # q4=0: 0-1, q4=1: 2-5, q4=2: 6-11, q4=3: 12-19) carries filler
            # units, each tagged with the earliest legal group.
            def build_plan(jobs, nslots=20, heat_from=20):
                """jobs: [(min_slot, max_slot, callable)] -> {slot: [fns]},
                greedily balancing load within each job's legal window.
                Empty slots get a heater; slots >= heat_from get one extra
                (late-pair insurance against clock-gate collapse)."""
                plan = collections.defaultdict(list)
                for ms, mx, fn in sorted(jobs, key=lambda j: (j[1], j[0])):
                    slot = min(range(ms, mx + 1),
                               key=lambda s_: (len(plan[s_]), s_))
                    plan[slot].append(fn)
                return plan

            for pr in range(PAIRS):
                jobs = []
                if pr == 0:
                    # rest of the V projection and own QK, due before the
                    # span that consumes them (q4=s starts at slot
                    # {1:2, 2:6, 3:12})
                    win = {1: (0, 1), 2: (2, 5), 3: (6, 11)}
                    for blk in (1, 2, 3):
                        lo, hi = win[blk]
                        jobs += [(lo, hi, lambda kb=kb: v_unit(kb))
                                 for kb in range(4 * blk, 4 * blk + 4)]
                        jobs += [(lo, hi,
                                  lambda t=t, blk=blk: qk_unit(0, t, blk))
                                 for t in (0, 1)]
                if pr + 1 < PAIRS:
                    # next pair's qk units (pair 2 keeps mc3 back: pair 3
                    # fills its own first groups with them)
                    jobs += [
                        (0, 19, lambda t=t, mc=mc: qk_unit(pr + 1, t, mc))
                        for mc in range(MC if pr < 2 else MC - 1)
                        for t in (0, 1)]
                else:
                    jobs += [(0, 0, lambda: qk_unit(3, 0, 3)),
                             (1, 1, lambda: qk_unit(3, 1, 3))]
                if pr == 1:
                    # out-proj part A (pairs 0+1) for spans this pair has
                    # normalized: span s legal once norm_b(p1, s) popped
                    # (q4=s+1, g>=2)
                    span_min = {0: 4, 1: 8}
                    for s in (0, 1):
                        jobs += [
                            (span_min[s], 19,
                             lambda qt=qt, fc=fc: oproj_a(qt, fc))
                            for qt in range(4 * s, 4 * s + 4)
                            for fc in (0, 1)]
                elif pr == 2:
                    # part A for spans 2-3; span 3 must wait for pair 1's
                    # deferred span-3 norms (popped at slots 0-1)
                    jobs += [
                        (0 if qt < 12 else 2, 19,
                         lambda qt=qt, fc=fc: oproj_a(qt, fc))
                        for qt in range(8, KB) for fc in (0, 1)]
                elif pr == 3:
                    # part B (all pairs) for spans this pair has finished
                    span_min = {0: 4, 1: 8, 2: 14}
                    for s in (0, 1, 2):
                        jobs += [
                            (span_min[s], 19,
                             lambda qt=qt, fc=fc: oproj_b(qt, fc))
                            for qt in range(4 * s, 4 * s + 4)
                            for fc in (0, 1)]
                attention_pair(pr, build_plan(jobs))

            # flush deferred normalizations, then the last token span's
            # out-projection part B
            while pending_norm:
                pending_norm.pop(0)()
            for qt in range(12, KB):
                for fc in (0, 1):
                    oproj_b(qt, fc)

    nc.compile()
    return nc


def make_core_inputs(x, Wqkv_w, Wqkv_b, out_w, out_b, H, n_tp):
    """Host-side shard + layout prep. Returns list of in_maps (one per core).
    Core c handles batch c // n_tp, head group c % n_tp."""
    B, L, D = x.shape
    hpg = H // n_tp            # heads per core
    EV = hpg * HD
    ET = 2 * hpg * HD // P
    bf = ml_dtypes.bfloat16
    tri = np.triu(np.ones((P, P), dtype=np.float32))  # [k, q]: 1 if q >= k
    in_maps = []
    for c in range(B * n_tp):
        b, g = c // n_tp, c % n_tp
        # qk row order: per pair p -> q(2p), q(2p+1), k(2p), k(2p+1)
        qk_rows = []
        for p_ in range(hpg // 2):
            for h in (2 * p_, 2 * p_ + 1):
                qk_rows.extend(range(g * hpg * HD + h * HD,
                                     g * hpg * HD + h * HD + HD))
            for h in (2 * p_, 2 * p_ + 1):
                qk_rows.extend(range(D + g * hpg * HD + h * HD,
                                     D + g * hpg * HD + h * HD + HD))
        qk_rows = np.array(qk_rows)
        v_rows = np.arange(2 * D + g * EV, 2 * D + (g + 1) * EV)
        in_maps.append({
            "xT": np.ascontiguousarray(x[b].T).astype(bf),
            "wqkT": np.ascontiguousarray(Wqkv_w[qk_rows].T).astype(bf),
            "wvT": np.ascontiguousarray(Wqkv_w[v_rows].T).astype(bf),
            "bqk": np.ascontiguousarray(
                Wqkv_b[qk_rows].reshape(ET, P).T).astype(np.float32),
            "vb": np.tile(Wqkv_b[v_rows], (P, 1)).astype(bf),
            "woT": np.ascontiguousarray(
                out_w[:, g * EV:(g + 1) * EV].T).astype(bf),
            "ob": (np.tile(out_b, (P, 1)).astype(np.float32) if g == 0
                   else np.zeros((P, D), np.float32)),
            "tri": tri.astype(bf),
            "onesd": np.ones((P, 512), np.float32).astype(bf),
        })
    return in_maps


_NC_CACHE = {}
LAST_RESULTS = None


def kernel(x, Wqkv_w, Wqkv_b, out_w, out_b):
    global LAST_RESULTS
    x = np.asarray(x, dtype=np.float32)
    Wqkv_w = np.asarray(Wqkv_w, dtype=np.float32)
    Wqkv_b = np.asarray(Wqkv_b, dtype=np.float32)
    out_w = np.asarray(out_w, dtype=np.float32)
    out_b = np.asarray(out_b, dtype=np.float32)

    B, L, D = x.shape
    H = 16
    n_tp = 2
    hpg = H // n_tp

    key = (L, D, hpg)
    if key not in _NC_CACHE:
        _NC_CACHE[key] = build_mha_nc(L, D, hpg)
    nc = _NC_CACHE[key]

    in_maps = make_core_inputs(x, Wqkv_w, Wqkv_b, out_w, out_b, H, n_tp)

    from concourse.bass_utils import run_bass_kernel_spmd
    res = run_bass_kernel_spmd(nc, in_maps, core_ids=list(range(len(in_maps))))
    LAST_RESULTS = res

    out = np.empty((B, L, D), dtype=np.float32)
    for b in range(B):
        out[b] = res.results[n_tp * b]["out"]
        for g in range(1, n_tp):
            out[b] += res.results[n_tp * b + g]["out"]
    return out


if __name__ == "__main__":
    nc = build_mha_nc(2048, 1024, 8)
    print("built OK")


# revision 56
# speedup vs baseline: 1.0541x; 1.0234x over previous
"""Trainium2 Bass kernel for causal MHA (B=4, L=2048, D=1024, H=16), 8 cores.

Sharding: data-parallel over batch (4) x tensor-parallel over heads (2).
Each core handles one batch element and 8 heads.

v2 design (vs baseline):
  - bf16 operands everywhere (PSUM accumulation stays fp32); rel-err budget
    is 2e-2 and bf16 lands ~2e-3.
  - Everything SBUF-resident: x, Q/K, V (with fused ones-row for the softmax
    denominator), attention output, all weights. No DRAM bounce for V.
  - Score matmuls for the two heads of a pair issue back-to-back at base
    partitions 0/64 so the K=64 pairs run concurrently in different PE row
    groups.
  - exp() issued 1024 wide (st PSUM tiles span 2 banks) to amortize ACT
    per-instruction overhead; diagonal groups split to skip masked columns.
  - QKV projection matmul chains for pair p+1 are interleaved into pair p's
    attention emission (and out-projection chains into pair 3's) so the
    tensor queue always has ready work: keeps the PE HAM clock warm.
  - Softmax denominators: ones-row of V gives den in PSUM row 64; DVE
    reciprocal on a DMA-folded [8,128] tile; K=1 broadcast matmul spreads
    1/den across 64 partitions for the normalization multiplies.
"""

import collections
import contextlib

import numpy as np
import ml_dtypes

import concourse.bass as bass
import concourse.bacc as bacc
import concourse.mybir as mybir
import concourse.tile as tile

P = 128
HD = 64  # head dim

F32 = mybir.dt.float32
BF16 = mybir.dt.bfloat16


def build_mha_nc(L, D, HEADS):
    """Build the per-core Bass program. One batch element, HEADS heads."""
    DBLK = D // P          # contraction blocks for projections (8)
    KB = L // P            # key blocks (16)
    MC = L // 512          # token chunks for projections (4)
    PAIRS = HEADS // 2     # head pairs (4)
    EV = HEADS * HD        # v channels per core (512)
    EQK = 2 * HEADS * HD   # q+k rows per core (1024)
    ET = EQK // P          # qk tiles: per pair, one q-tile + one k-tile (8)
    NQ = L // 512          # query spans (4)
    scale = 1.0 / float(np.sqrt(HD))

    nc = bacc.Bacc("TRN2", target_bir_lowering=False, debug=False,
                   enable_asserts=False)

    xT = nc.dram_tensor("xT", [D, L], BF16, kind="ExternalInput").ap()
    wqkT = nc.dram_tensor("wqkT", [D, EQK], BF16, kind="ExternalInput").ap()
    wvT = nc.dram_tensor("wvT", [D, EV], BF16, kind="ExternalInput").ap()
    bqk = nc.dram_tensor("bqk", [P, ET], F32, kind="ExternalInput").ap()
    vb = nc.dram_tensor("vb", [P, EV], BF16, kind="ExternalInput").ap()
    woT = nc.dram_tensor("woT", [EV, D], BF16, kind="ExternalInput").ap()
    ob = nc.dram_tensor("ob", [P, D], F32, kind="ExternalInput").ap()
    tri = nc.dram_tensor("tri", [P, P], BF16, kind="ExternalInput").ap()
    onesd = nc.dram_tensor("onesd", [P, 512], BF16, kind="ExternalInput").ap()
    out = nc.dram_tensor("out", [L, D], F32, kind="ExternalOutput").ap()

    with tile.TileContext(nc) as tc:
        ctx = contextlib.ExitStack()
        with ctx:
            consts = ctx.enter_context(tc.tile_pool(name="consts", bufs=1))
            qk_pool = ctx.enter_context(tc.tile_pool(name="qk", bufs=6))
            ex_pool = ctx.enter_context(tc.tile_pool(name="ex", bufs=10))
            drow_pool = ctx.enter_context(tc.tile_pool(name="drow", bufs=3))
            tmp_pool = ctx.enter_context(tc.tile_pool(name="tmp", bufs=3))
            den_pool = ctx.enter_context(tc.tile_pool(name="den", bufs=3))
            recl_pool = ctx.enter_context(tc.tile_pool(name="recl", bufs=3))
            outst_pool = ctx.enter_context(tc.tile_pool(name="outst", bufs=8))
            st_ps = ctx.enter_context(
                tc.tile_pool(name="st_ps", bufs=2, space="PSUM"))
            av_ps = ctx.enter_context(
                tc.tile_pool(name="av_ps", bufs=2, space="PSUM"))
            mm_ps = ctx.enter_context(
                tc.tile_pool(name="mm_ps", bufs=2, space="PSUM"))

            # ---- SBUF-resident tensors ----
            tri_sb = consts.tile([P, P], BF16, name="tri_sb")
            ones_sb = consts.tile([P, 512], BF16, name="ones_sb")
            bqk_sb = consts.tile([P, ET], F32, name="bqk_sb")
            vb_sb = consts.tile([P, EV], BF16, name="vb_sb")
            ob_sb = consts.tile([P, D], F32, name="ob_sb")
            x_sb = consts.tile([P, DBLK, L], BF16, name="x_sb")
            wqk_sb = consts.tile([P, DBLK, EQK], BF16, name="wqk_sb")
            wv_sb = consts.tile([P, DBLK, EV], BF16, name="wv_sb")
            wo_sb = consts.tile([P, EV // P, D], BF16, name="wo_sb")
            v_all = consts.tile([P, KB, HEADS, HD + 1], BF16, name="v_all")
            attn_sb = consts.tile([P, PAIRS, L], BF16, name="attn_sb")
            # partial out-projection accumulator (pairs 0+1 contribution,
            # bias included) — lets half the out-proj run as filler during
            # pairs 1-2's attention
            ypart = consts.tile([P, KB, 2, 512], BF16, name="ypart")

            # small consts on the gpsimd queue; ones first (heater dep)
            nc.gpsimd.dma_start(out=ones_sb, in_=onesd)
            nc.gpsimd.dma_start(out=tri_sb, in_=tri)
            nc.gpsimd.dma_start(out=bqk_sb, in_=bqk)
            nc.gpsimd.dma_start(out=vb_sb, in_=vb)
            nc.gpsimd.dma_start(out=ob_sb, in_=ob)

            # big loads, chunked so first matmuls can start early; weights
            # on the scalar queue run parallel to x on the sync queue
            _wv_src = wvT.rearrange("(o p) e -> p o e", p=P)
            for o in range(0, DBLK, DBLK // 2):
                nc.scalar.dma_start(out=wv_sb[:, o:o + DBLK // 2, :],
                                    in_=_wv_src[:, o:o + DBLK // 2, :])
            _wqk_src = wqkT.rearrange("(o p) e -> p o e", p=P)
            _wo_src = woT.rearrange("(j p) f -> p j f", p=P)
            # all weight slices up front (pair 0 first) so attention-phase
            # filler units never stall on an in-flight weight DMA
            for pr_ in range(PAIRS):
                nc.scalar.dma_start(
                    out=wqk_sb[:, :, pr_ * 2 * P:(pr_ + 1) * 2 * P],
                    in_=_wqk_src[:, :, pr_ * 2 * P:(pr_ + 1) * 2 * P])
            for j in range(0, EV // P, 2):
                nc.scalar.dma_start(out=wo_sb[:, j:j + 2, :],
                                    in_=_wo_src[:, j:j + 2, :])
            _x_src = xT.rearrange("(o p) m -> p o m", p=P)
            for mc in range(MC):
                nc.sync.dma_start(
                    out=x_sb[:, :, mc * 512:(mc + 1) * 512],
                    in_=_x_src[:, :, mc * 512:(mc + 1) * 512])

            # ones column of V (softmax denominator trick)
            nc.vector.memset(v_all[:, :, :, HD:HD + 1], 1.0)

            # PE heater: junk matmuls during the startup DMA window so the
            # HAM clock gate is released (K=8/8) before real work arrives.
            # Results are never read.
            heat_ps = mm_ps.tile([P, 512], F32, name="heat_ps", tag="mm")
            for _ in range(36):
                nc.tensor.matmul(heat_ps, lhsT=ones_sb[:, 0:P], rhs=ones_sb,
                                 start=True, stop=True)

            qk_tiles = {}
            heat_n = [0]
            pin_after = [None]  # slot anchor: stops filler-unit hoisting

            def pin(mm):
                # anchor heaters to their slot (they have no natural deps
                # and would otherwise be hoisted early by the scheduler)
                if pin_after[0] is not None:
                    tile.add_dep_helper(mm.ins, pin_after[0].ins, sync=False,
                                        reason="pin heater to its slot")
                return mm

            def heater_unit(n=6):
                # junk matmuls: keeps the PE activity monitor busy through
                # slots with no real filler (prevents clock-gate collapse)
                heat_n[0] += 1
                hp = mm_ps.tile([P, 512], F32, name=f"heatu_{heat_n[0]}",
                                tag="mm")
                for k in range(n):
                    mm = nc.tensor.matmul(hp, lhsT=ones_sb[:, 0:P],
                                          rhs=ones_sb, start=True, stop=True)
                    if k == 0:
                        pin(mm)

            # ---------- work units (each: one PSUM accumulation chain) ----
            def v_unit(kb):
                ps = mm_ps.tile([P, 512], F32, name=f"vps_{kb}", tag="mm")
                for o in range(DBLK):
                    mm = nc.tensor.matmul(
                        ps,
                        lhsT=x_sb[:, o, kb * P:(kb + 1) * P],
                        rhs=wv_sb[:, o, :],
                        start=(o == 0), stop=(o == DBLK - 1))
                nc.vector.tensor_add(
                    out=v_all[:, kb, :, 0:HD],
                    in0=ps.rearrange("p (h c) -> p h c", c=HD),
                    in1=vb_sb.rearrange("p (h c) -> p h c", c=HD))

            def qk_unit(pr, t, mc):
                et = 2 * pr + t
                key = (pr, t)
                if key not in qk_tiles:
                    qk_tiles[key] = qk_pool.tile([P, L], BF16,
                                                 name=f"qk_{pr}_{t}", tag="qk")
                ps = mm_ps.tile([P, 512], F32, name=f"qkps_{et}_{mc}",
                                tag="mm")
                for o in range(DBLK):
                    mm = nc.tensor.matmul(
                        ps,
                        lhsT=wqk_sb[:, o, et * P:(et + 1) * P],
                        rhs=x_sb[:, o, mc * 512:(mc + 1) * 512],
                        start=(o == 0), stop=(o == DBLK - 1))
                nc.vector.tensor_scalar(
                    out=qk_tiles[key][:, mc * 512:(mc + 1) * 512],
                    in0=ps, scalar1=bqk_sb[:, et:et + 1], scalar2=None,
                    op0=mybir.AluOpType.add)

            def oproj_a(qt, fc):
                # pairs 0+1 contribution + bias -> ypart (bf16)
                f0 = fc * 512
                ps = mm_ps.tile([P, 512], F32, name=f"opa_{qt}_{fc}",
                                tag="mm")
                for j in (0, 1):
                    mm = nc.tensor.matmul(
                        ps,
                        lhsT=attn_sb[:, j, qt * P:(qt + 1) * P],
                        rhs=wo_sb[:, j, f0:f0 + 512],
                        start=(j == 0), stop=(j == 1))
                nc.vector.tensor_add(out=ypart[:, qt, fc, :], in0=ps,
                                     in1=ob_sb[:, f0:f0 + 512])

            def oproj_b(qt, fc):
                # pairs 2+3 contribution + ypart -> out (f32)
                f0 = fc * 512
                ps = mm_ps.tile([P, 512], F32, name=f"opb_{qt}_{fc}",
                                tag="mm")
                for j in (2, 3):
                    mm = nc.tensor.matmul(
                        ps,
                        lhsT=attn_sb[:, j, qt * P:(qt + 1) * P],
                        rhs=wo_sb[:, j, f0:f0 + 512],
                        start=(j == 2), stop=(j == 3))
                ot = outst_pool.tile([P, 512], F32, name=f"ot_{qt}_{fc}",
                                     tag="outst")
                nc.vector.tensor_add(out=ot, in0=ps, in1=ypart[:, qt, fc, :])
                nc.sync.dma_start(
                    out=out[qt * P:(qt + 1) * P, f0:f0 + 512], in_=ot)

            # ---------- attention for one pair, with filler interleave ----
            # Deferred softmax-normalization stages. The reciprocal/broadcast
            # chain of token span s is emitted one span LATER (between that
            # span's score groups) so it never sits ahead of ready matmuls
            # in the tensor queue. pending_norm carries stages across q4 and
            # pair boundaries.
            pending_norm = []

            def attention_pair(pr, fill_plan):
                """fill_plan: {sequential group idx -> [callables]} of
                projection / out-proj units, emitted between a group's exps
                and its AV matmuls so the PE has independent work while the
                ACT engine runs the exps."""
                q_tile = qk_tiles[(pr, 0)]
                k_tile = qk_tiles[(pr, 1)]
                gidx = 0
                pending_avs = [None]
                for q4 in range(NQ):
                    q0 = q4 * 512
                    nkb = 4 * (q4 + 1)
                    G = nkb // 2
                    avs = []
                    for hh in (0, 1):
                        av = av_ps.tile([HD + 1, 512], F32,
                                        name=f"av_{pr}_{hh}_{q4}", tag="av")
                        avs.append(av)
                    for g in range(G):
                        kbs = (2 * g, 2 * g + 1)
                        # kb-major tiles: each holds BOTH heads for one kb
                        # ([h0 | h1] across 2 banks) so exp fires after just
                        # the two concurrent score matmuls of that kb
                        sts = []
                        exs = []
                        for i, kb in enumerate(kbs):
                            s0 = max(0, kb * P - q0)
                            st = st_ps.tile([P, 1024], F32,
                                            name=f"st_{pr}_{q4}_{g}_{i}",
                                            tag="st")
                            # h0/h1 back-to-back: K=64 pairs run in
                            # different PE row groups concurrently
                            pair_mms = []
                            for hh in (0, 1):
                                rows = slice(hh * HD, hh * HD + HD)
                                pair_mms.append(nc.tensor.matmul(
                                    st[:, hh * 512 + s0:hh * 512 + 512],
                                    lhsT=k_tile[rows, kb * P:(kb + 1) * P],
                                    rhs=q_tile[rows, q0 + s0:q0 + 512],
                                    start=True, stop=True))
                            tile.add_dep_helper(
                                pair_mms[1].ins, pair_mms[0].ins, sync=False,
                                reason="keep score pair adjacent")
                            pin_after[0] = pair_mms[0]
                            sts.append(st)
                            # exp: one wide call unless the diagonal trim
                            # makes two narrow calls cheaper
                            ex = ex_pool.tile([P, 1024], BF16,
                                              name=f"ex_{pr}_{q4}_{g}_{i}",
                                              tag="ex")
                            if s0 <= 2 * P:
                                nc.scalar.activation(
                                    out=ex[:, s0:1024], in_=st[:, s0:1024],
                                    func=mybir.ActivationFunctionType.Exp,
                                    scale=scale)
                            else:
                                nc.scalar.activation(
                                    out=ex[:, s0:512], in_=st[:, s0:512],
                                    func=mybir.ActivationFunctionType.Exp,
                                    scale=scale)
                                nc.scalar.activation(
                                    out=ex[:, 512 + s0:1024],
                                    in_=st[:, 512 + s0:1024],
                                    func=mybir.ActivationFunctionType.Exp,
                                    scale=scale)
                            # causal mask on the diagonal block (gpsimd:
                            # off the busier vector queue)
                            d0 = kb * P - q0
                            if d0 >= 0:
                                for hh in (0, 1):
                                    c = hh * 512 + d0
                                    nc.gpsimd.tensor_mul(
                                        out=ex[:, c:c + P],
                                        in0=ex[:, c:c + P], in1=tri_sb)
                            exs.append(ex)
                        # previous group's AVs: their exps finished a
                        # full group ago, so they issue with zero wait and
                        # never split this group's concurrent score pairs
                        if pending_avs[0] is not None:
                            pending_avs[0]()

                        def emit_avs(exs=exs, kbs=kbs, q0=q0, nkb=nkb,
                                     avs=avs, pr=pr):
                            for i, kb in enumerate(kbs):
                                s0 = max(0, kb * P - q0)
                                for hh in (0, 1):
                                    h = 2 * pr + hh
                                    nc.tensor.matmul(
                                        avs[hh][:, s0:512],
                                        lhsT=v_all[:, kb, h, :],
                                        rhs=exs[i][:, hh * 512 + s0:
                                                   hh * 512 + 512],
                                        start=(kb == 0),
                                        stop=(kb == nkb - 1))
                        pending_avs[0] = emit_avs
                        # deferred norms + filler cover the ACT latency
                        if g <= 1 and pending_norm:
                            pending_norm.pop(0)()
                        for fn in fill_plan.get(gidx, ()):
                            fn()
                        gidx += 1
                    # flush the last group's AVs before the epilogue
                    pending_avs[0]()
                    pending_avs[0] = None

                    # ---- q4 epilogue: evacuate now, normalize deferred ----
                    dr = drow_pool.tile([P, 512], BF16, name=f"dr_{pr}_{q4}",
                                        tag="drow")
                    tmpt = tmp_pool.tile([P, 512], BF16, name=f"tp_{pr}_{q4}",
                                         tag="tmp")
                    den = den_pool.tile([8, P], BF16, name=f"den_{pr}_{q4}",
                                        tag="den")
                    recl = recl_pool.tile([P, 2, 512], BF16,
                                          name=f"recl_{pr}_{q4}", tag="recl")
                    # h0: raw attn to attn_sb, den row to dr
                    nc.vector.tensor_copy(
                        out=attn_sb[0:HD, pr, q0:q0 + 512], in_=avs[0][0:HD, :])
                    nc.vector.tensor_copy(out=dr[HD:HD + 1, :],
                                          in_=avs[0][HD:HD + 1, :])
                    # h1: raw attn + den row to tmp
                    nc.vector.tensor_copy(out=tmpt[0:HD + 1, :], in_=avs[1])
                    # fold den rows [1,512] -> [4,128] for a cheap reciprocal
                    nc.gpsimd.dma_start(out=den[0:4, :], in_=dr[HD:HD + 1, :])
                    nc.gpsimd.dma_start(out=den[4:8, :],
                                        in_=tmpt[HD:HD + 1, :])

                    def norm_a(den=den, recl=recl):
                        with nc.allow_low_precision(
                                reason="bf16 rounding of softmax denom"):
                            nc.vector.reciprocal(out=den, in_=den)
                        # unfold to partition 64 (legal matmul base partition)
                        nc.gpsimd.dma_start(out=recl[HD:HD + 1, 0, :],
                                            in_=den[0:4, :])
                        nc.gpsimd.dma_start(out=recl[HD:HD + 1, 1, :],
                                            in_=den[4:8, :])

                    def norm_b(pr=pr, q0=q0, recl=recl, tmpt=tmpt):
                        # broadcast 1/den across 64 partitions via K=1 matmul
                        bpst = st_ps.tile([P, 1024], F32,
                                          name=f"bps_{pr}_{q0}", tag="st")
                        for hh in (0, 1):
                            nc.tensor.matmul(
                                bpst[0:HD, hh * 512:hh * 512 + 512],
                                lhsT=ones_sb[HD:HD + 1, 0:HD],
                                rhs=recl[HD:HD + 1, hh, :],
                                start=True, stop=True)
                        sl = attn_sb[0:HD, pr, q0:q0 + 512]
                        nc.vector.tensor_mul(out=sl, in0=sl,
                                             in1=bpst[0:HD, 0:512])
                        nc.vector.tensor_mul(out=tmpt[0:HD, :],
                                             in0=tmpt[0:HD, :],
                                             in1=bpst[0:HD, 512:1024])
                        nc.gpsimd.dma_start(
                            out=attn_sb[HD:P, pr, q0:q0 + 512],
                            in_=tmpt[0:HD, :])

                    pending_norm.append(norm_a)
                    pending_norm.append(norm_b)

            # ---------- emission schedule ----------
            # Minimal prologue: just what pair-0 span 0 needs. The rest of
            # the V/QK0 projections ride inside pair 0's attention as
            # deadline-constrained filler — this keeps the attention region
            # PE-bound (ACT would otherwise dominate it and the PE clock
            # gate would oscillate).
            for kb in range(4):
                v_unit(kb)
            qk_unit(0, 0, 0)
            qk_unit(0, 1, 0)

            # Pairs: each pair's attention (20 groups, indices 0..19 —
            # q4=0: 0-1, q4=1: 2-5, q4=2: 6-11, q4=3: 12-19) carries filler
            # units, each tagged with the earliest legal group.
            def build_plan(jobs, nslots=20, heat_from=20):
                """jobs: [(min_slot, max_slot, callable)] -> {slot: [fns]},
                greedily balancing load within each job's legal window.
                Empty slots get a heater; slots >= heat_from get one extra
                (late-pair insurance against clock-gate collapse)."""
                plan = collections.defaultdict(list)
                for ms, mx, fn in sorted(jobs, key=lambda j: (j[1], j[0])):
                    slot = min(range(ms, mx + 1),
                               key=lambda s_: (len(plan[s_]), s_))
                    plan[slot].append(fn)
                return plan

            for pr in range(PAIRS):
                jobs = []
                if pr == 0:
                    # rest of the V projection and own QK, due before the
                    # span that consumes them (q4=s starts at slot
                    # {1:2, 2:6, 3:12})
                    win = {1: (0, 1), 2: (2, 5), 3: (6, 11)}
                    for blk in (1, 2, 3):
                        lo, hi = win[blk]
                        jobs += [(lo, hi, lambda kb=kb: v_unit(kb))
                                 for kb in range(4 * blk, 4 * blk + 4)]
                        jobs += [(lo, hi,
                                  lambda t=t, blk=blk: qk_unit(0, t, blk))
                                 for t in (0, 1)]
                if pr + 1 < PAIRS:
                    # next pair's qk units (pair 2 keeps mc3 back: pair 3
                    # fills its own first groups with them)
                    jobs += [
                        (0, 19, lambda t=t, mc=mc: qk_unit(pr + 1, t, mc))
                        for mc in range(MC if pr < 2 else MC - 1)
                        for t in (0, 1)]
                else:
                    jobs += [(0, 0, lambda: qk_unit(3, 0, 3)),
                             (1, 1, lambda: qk_unit(3, 1, 3))]
                if pr == 1:
                    # out-proj part A (pairs 0+1) for spans this pair has
                    # normalized: span s legal once norm_b(p1, s) popped
                    # (q4=s+1, g>=2)
                    span_min = {0: 4, 1: 8}
                    for s in (0, 1):
                        jobs += [
                            (span_min[s], 19,
                             lambda qt=qt, fc=fc: oproj_a(qt, fc))
                            for qt in range(4 * s, 4 * s + 4)
                            for fc in (0, 1)]
                elif pr == 2:
                    # part A for spans 2-3; span 3 must wait for pair 1's
                    # deferred span-3 norms (popped at slots 0-1)
                    jobs += [
                        (0 if qt < 12 else 2, 19,
                         lambda qt=qt, fc=fc: oproj_a(qt, fc))
                        for qt in range(8, KB) for fc in (0, 1)]
                elif pr == 3:
                    # part B (all pairs) for spans this pair has finished
                    span_min = {0: 4, 1: 8, 2: 14}
                    for s in (0, 1, 2):
                        jobs += [
                            (span_min[s], 19,
                             lambda qt=qt, fc=fc: oproj_b(qt, fc))
                            for qt in range(4 * s, 4 * s + 4)
                            for fc in (0, 1)]
                attention_pair(pr, build_plan(jobs))

            # flush deferred normalizations, then the last token span's
            # out-projection part B
            while pending_norm:
                pending_norm.pop(0)()
            for qt in range(12, KB):
                for fc in (0, 1):
                    oproj_b(qt, fc)

    nc.compile()
    return nc


def make_core_inputs(x, Wqkv_w, Wqkv_b, out_w, out_b, H, n_tp):
    """Host-side shard + layout prep. Returns list of in_maps (one per core).
    Core c handles batch c // n_tp, head group c % n_tp."""
    B, L, D = x.shape
    hpg = H // n_tp            # heads per core
    EV = hpg * HD
    ET = 2 * hpg * HD // P
    bf = ml_dtypes.bfloat16
    tri = np.triu(np.ones((P, P), dtype=np.float32))  # [k, q]: 1 if q >= k
    in_maps = []
    for c in range(B * n_tp):
        b, g = c // n_tp, c % n_tp
        # qk row order: per pair p -> q(2p), q(2p+1), k(2p), k(2p+1)
        qk_rows = []
        for p_ in range(hpg // 2):
            for h in (2 * p_, 2 * p_ + 1):
                qk_rows.extend(range(g * hpg * HD + h * HD,
                                     g * hpg * HD + h * HD + HD))
            for h in (2 * p_, 2 * p_ + 1):
                qk_rows.extend(range(D + g * hpg * HD + h * HD,
                                     D + g * hpg * HD + h * HD + HD))
        qk_rows = np.array(qk_rows)
        v_rows = np.arange(2 * D + g * EV, 2 * D + (g + 1) * EV)
        in_maps.append({
            "xT": np.ascontiguousarray(x[b].T).astype(bf),
            "wqkT": np.ascontiguousarray(Wqkv_w[qk_rows].T).astype(bf),
            "wvT": np.ascontiguousarray(Wqkv_w[v_rows].T).astype(bf),
            "bqk": np.ascontiguousarray(
                Wqkv_b[qk_rows].reshape(ET, P).T).astype(np.float32),
            "vb": np.tile(Wqkv_b[v_rows], (P, 1)).astype(bf),
            "woT": np.ascontiguousarray(
                out_w[:, g * EV:(g + 1) * EV].T).astype(bf),
            "ob": (np.tile(out_b, (P, 1)).astype(np.float32) if g == 0
                   else np.zeros((P, D), np.float32)),
            "tri": tri.astype(bf),
            "onesd": np.ones((P, 512), np.float32).astype(bf),
        })
    return in_maps


_NC_CACHE = {}
LAST_RESULTS = None


def kernel(x, Wqkv_w, Wqkv_b, out_w, out_b):
    global LAST_RESULTS
    x = np.asarray(x, dtype=np.float32)
    Wqkv_w = np.asarray(Wqkv_w, dtype=np.float32)
    Wqkv_b = np.asarray(Wqkv_b, dtype=np.float32)
    out_w = np.asarray(out_w, dtype=np.float32)
    out_b = np.asarray(out_b, dtype=np.float32)

    B, L, D = x.shape
    H = 16
    n_tp = 2
    hpg = H // n_tp

    key = (L, D, hpg)
    if key not in _NC_CACHE:
        _NC_CACHE[key] = build_mha_nc(L, D, hpg)
    nc = _NC_CACHE[key]

    in_maps = make_core_inputs(x, Wqkv_w, Wqkv_b, out_w, out_b, H, n_tp)

    from concourse.bass_utils import run_bass_kernel_spmd
    res = run_bass_kernel_spmd(nc, in_maps, core_ids=list(range(len(in_maps))))
    LAST_RESULTS = res

    out = np.empty((B, L, D), dtype=np.float32)
    for b in range(B):
        out[b] = res.results[n_tp * b]["out"]
        for g in range(1, n_tp):
            out[b] += res.results[n_tp * b + g]["out"]
    return out


if __name__ == "__main__":
    nc = build_mha_nc(2048, 1024, 8)
    print("built OK")


# revision 57
# speedup vs baseline: 1.0747x; 1.0195x over previous
"""Trainium2 Bass kernel for causal MHA (B=4, L=2048, D=1024, H=16), 8 cores.

Sharding: data-parallel over batch (4) x tensor-parallel over heads (2).
Each core handles one batch element and 8 heads.

v2 design (vs baseline):
  - bf16 operands everywhere (PSUM accumulation stays fp32); rel-err budget
    is 2e-2 and bf16 lands ~2e-3.
  - Everything SBUF-resident: x, Q/K, V (with fused ones-row for the softmax
    denominator), attention output, all weights. No DRAM bounce for V.
  - Score matmuls for the two heads of a pair issue back-to-back at base
    partitions 0/64 so the K=64 pairs run concurrently in different PE row
    groups.
  - exp() issued 1024 wide (st PSUM tiles span 2 banks) to amortize ACT
    per-instruction overhead; diagonal groups split to skip masked columns.
  - QKV projection matmul chains for pair p+1 are interleaved into pair p's
    attention emission (and out-projection chains into pair 3's) so the
    tensor queue always has ready work: keeps the PE HAM clock warm.
  - Softmax denominators: ones-row of V gives den in PSUM row 64; DVE
    reciprocal on a DMA-folded [8,128] tile; K=1 broadcast matmul spreads
    1/den across 64 partitions for the normalization multiplies.
"""

import collections
import contextlib

import numpy as np
import ml_dtypes

import concourse.bass as bass
import concourse.bacc as bacc
import concourse.mybir as mybir
import concourse.tile as tile

P = 128
HD = 64  # head dim

F32 = mybir.dt.float32
BF16 = mybir.dt.bfloat16


def build_mha_nc(L, D, HEADS):
    """Build the per-core Bass program. One batch element, HEADS heads."""
    DBLK = D // P          # contraction blocks for projections (8)
    KB = L // P            # key blocks (16)
    MC = L // 512          # token chunks for projections (4)
    PAIRS = HEADS // 2     # head pairs (4)
    EV = HEADS * HD        # v channels per core (512)
    EQK = 2 * HEADS * HD   # q+k rows per core (1024)
    ET = EQK // P          # qk tiles: per pair, one q-tile + one k-tile (8)
    NQ = L // 512          # query spans (4)
    scale = 1.0 / float(np.sqrt(HD))

    nc = bacc.Bacc("TRN2", target_bir_lowering=False, debug=False,
                   enable_asserts=False)

    xT = nc.dram_tensor("xT", [D, L], BF16, kind="ExternalInput").ap()
    wqkT = nc.dram_tensor("wqkT", [D, EQK], BF16, kind="ExternalInput").ap()
    wvT = nc.dram_tensor("wvT", [D, EV], BF16, kind="ExternalInput").ap()
    bqk = nc.dram_tensor("bqk", [P, ET], F32, kind="ExternalInput").ap()
    vb = nc.dram_tensor("vb", [P, EV], BF16, kind="ExternalInput").ap()
    woT = nc.dram_tensor("woT", [EV, D], BF16, kind="ExternalInput").ap()
    ob = nc.dram_tensor("ob", [P, D], F32, kind="ExternalInput").ap()
    tri = nc.dram_tensor("tri", [P, P], BF16, kind="ExternalInput").ap()
    onesd = nc.dram_tensor("onesd", [P, 512], BF16, kind="ExternalInput").ap()
    out = nc.dram_tensor("out", [L, D], F32, kind="ExternalOutput").ap()

    with tile.TileContext(nc) as tc:
        ctx = contextlib.ExitStack()
        with ctx:
            consts = ctx.enter_context(tc.tile_pool(name="consts", bufs=1))
            qk_pool = ctx.enter_context(tc.tile_pool(name="qk", bufs=6))
            ex_pool = ctx.enter_context(tc.tile_pool(name="ex", bufs=10))
            drow_pool = ctx.enter_context(tc.tile_pool(name="drow", bufs=3))
            tmp_pool = ctx.enter_context(tc.tile_pool(name="tmp", bufs=3))
            den_pool = ctx.enter_context(tc.tile_pool(name="den", bufs=3))
            recl_pool = ctx.enter_context(tc.tile_pool(name="recl", bufs=3))
            outst_pool = ctx.enter_context(tc.tile_pool(name="outst", bufs=8))
            st_ps = ctx.enter_context(
                tc.tile_pool(name="st_ps", bufs=2, space="PSUM"))
            av_ps = ctx.enter_context(
                tc.tile_pool(name="av_ps", bufs=2, space="PSUM"))
            mm_ps = ctx.enter_context(
                tc.tile_pool(name="mm_ps", bufs=2, space="PSUM"))

            # ---- SBUF-resident tensors ----
            tri_sb = consts.tile([P, P], BF16, name="tri_sb")
            ones_sb = consts.tile([P, 512], BF16, name="ones_sb")
            bqk_sb = consts.tile([P, ET], F32, name="bqk_sb")
            vb_sb = consts.tile([P, EV], BF16, name="vb_sb")
            ob_sb = consts.tile([P, D], F32, name="ob_sb")
            x_sb = consts.tile([P, DBLK, L], BF16, name="x_sb")
            wqk_sb = consts.tile([P, DBLK, EQK], BF16, name="wqk_sb")
            wv_sb = consts.tile([P, DBLK, EV], BF16, name="wv_sb")
            wo_sb = consts.tile([P, EV // P, D], BF16, name="wo_sb")
            v_all = consts.tile([P, KB, HEADS, HD + 1], BF16, name="v_all")
            attn_sb = consts.tile([P, PAIRS, L], BF16, name="attn_sb")
            # partial out-projection accumulator (pairs 0+1 contribution,
            # bias included) — lets half the out-proj run as filler during
            # pairs 1-2's attention
            ypart = consts.tile([P, KB, 2, 512], BF16, name="ypart")

            # small consts on the gpsimd queue; ones first (heater dep)
            nc.gpsimd.dma_start(out=ones_sb, in_=onesd)
            nc.gpsimd.dma_start(out=tri_sb, in_=tri)
            nc.gpsimd.dma_start(out=bqk_sb, in_=bqk)
            nc.gpsimd.dma_start(out=vb_sb, in_=vb)
            nc.gpsimd.dma_start(out=ob_sb, in_=ob)

            # big loads, chunked so first matmuls can start early; weights
            # on the scalar queue run parallel to x on the sync queue
            _wv_src = wvT.rearrange("(o p) e -> p o e", p=P)
            for o in range(0, DBLK, DBLK // 2):
                nc.scalar.dma_start(out=wv_sb[:, o:o + DBLK // 2, :],
                                    in_=_wv_src[:, o:o + DBLK // 2, :])
            _wqk_src = wqkT.rearrange("(o p) e -> p o e", p=P)
            _wo_src = woT.rearrange("(j p) f -> p j f", p=P)
            # all weight slices up front (pair 0 first) so attention-phase
            # filler units never stall on an in-flight weight DMA
            for pr_ in range(PAIRS):
                nc.scalar.dma_start(
                    out=wqk_sb[:, :, pr_ * 2 * P:(pr_ + 1) * 2 * P],
                    in_=_wqk_src[:, :, pr_ * 2 * P:(pr_ + 1) * 2 * P])
            for j in range(0, EV // P, 2):
                nc.scalar.dma_start(out=wo_sb[:, j:j + 2, :],
                                    in_=_wo_src[:, j:j + 2, :])
            _x_src = xT.rearrange("(o p) m -> p o m", p=P)
            for mc in range(MC):
                nc.sync.dma_start(
                    out=x_sb[:, :, mc * 512:(mc + 1) * 512],
                    in_=_x_src[:, :, mc * 512:(mc + 1) * 512])

            # ones column of V (softmax denominator trick)
            nc.vector.memset(v_all[:, :, :, HD:HD + 1], 1.0)

            # PE heater: junk matmuls during the startup DMA window so the
            # HAM clock gate is released (K=8/8) before real work arrives.
            # Results are never read.
            heat_ps = mm_ps.tile([P, 512], F32, name="heat_ps", tag="mm")
            for _ in range(36):
                nc.tensor.matmul(heat_ps, lhsT=ones_sb[:, 0:P], rhs=ones_sb,
                                 start=True, stop=True)

            qk_tiles = {}
            heat_n = [0]
            pin_after = [None]  # slot anchor: stops filler-unit hoisting

            def pin(mm):
                # anchor heaters to their slot (they have no natural deps
                # and would otherwise be hoisted early by the scheduler)
                if pin_after[0] is not None:
                    tile.add_dep_helper(mm.ins, pin_after[0].ins, sync=False,
                                        reason="pin heater to its slot")
                return mm

            def heater_unit(n=6):
                # junk matmuls: keeps the PE activity monitor busy through
                # slots with no real filler (prevents clock-gate collapse)
                heat_n[0] += 1
                hp = mm_ps.tile([P, 512], F32, name=f"heatu_{heat_n[0]}",
                                tag="mm")
                for k in range(n):
                    mm = nc.tensor.matmul(hp, lhsT=ones_sb[:, 0:P],
                                          rhs=ones_sb, start=True, stop=True)
                    if k == 0:
                        pin(mm)

            # ---------- work units (each: one PSUM accumulation chain) ----
            def v_unit(kb):
                ps = mm_ps.tile([P, 512], F32, name=f"vps_{kb}", tag="mm")
                for o in range(DBLK):
                    mm = nc.tensor.matmul(
                        ps,
                        lhsT=x_sb[:, o, kb * P:(kb + 1) * P],
                        rhs=wv_sb[:, o, :],
                        start=(o == 0), stop=(o == DBLK - 1))
                nc.vector.tensor_add(
                    out=v_all[:, kb, :, 0:HD],
                    in0=ps.rearrange("p (h c) -> p h c", c=HD),
                    in1=vb_sb.rearrange("p (h c) -> p h c", c=HD))

            def qk_unit(pr, t, mc):
                et = 2 * pr + t
                key = (pr, t)
                if key not in qk_tiles:
                    qk_tiles[key] = qk_pool.tile([P, L], BF16,
                                                 name=f"qk_{pr}_{t}", tag="qk")
                ps = mm_ps.tile([P, 512], F32, name=f"qkps_{et}_{mc}",
                                tag="mm")
                for o in range(DBLK):
                    mm = nc.tensor.matmul(
                        ps,
                        lhsT=wqk_sb[:, o, et * P:(et + 1) * P],
                        rhs=x_sb[:, o, mc * 512:(mc + 1) * 512],
                        start=(o == 0), stop=(o == DBLK - 1))
                nc.vector.tensor_scalar(
                    out=qk_tiles[key][:, mc * 512:(mc + 1) * 512],
                    in0=ps, scalar1=bqk_sb[:, et:et + 1], scalar2=None,
                    op0=mybir.AluOpType.add)

            def oproj_a(qt, fc):
                # pairs 0+1 contribution + bias -> ypart (bf16)
                f0 = fc * 512
                ps = mm_ps.tile([P, 512], F32, name=f"opa_{qt}_{fc}",
                                tag="mm")
                for j in (0, 1):
                    mm = nc.tensor.matmul(
                        ps,
                        lhsT=attn_sb[:, j, qt * P:(qt + 1) * P],
                        rhs=wo_sb[:, j, f0:f0 + 512],
                        start=(j == 0), stop=(j == 1))
                nc.vector.tensor_add(out=ypart[:, qt, fc, :], in0=ps,
                                     in1=ob_sb[:, f0:f0 + 512])

            def oproj_b(qt, fc):
                # pairs 2+3 contribution + ypart -> out (f32)
                f0 = fc * 512
                ps = mm_ps.tile([P, 512], F32, name=f"opb_{qt}_{fc}",
                                tag="mm")
                for j in (2, 3):
                    mm = nc.tensor.matmul(
                        ps,
                        lhsT=attn_sb[:, j, qt * P:(qt + 1) * P],
                        rhs=wo_sb[:, j, f0:f0 + 512],
                        start=(j == 2), stop=(j == 3))
                ot = outst_pool.tile([P, 512], F32, name=f"ot_{qt}_{fc}",
                                     tag="outst")
                nc.vector.tensor_add(out=ot, in0=ps, in1=ypart[:, qt, fc, :])
                nc.sync.dma_start(
                    out=out[qt * P:(qt + 1) * P, f0:f0 + 512], in_=ot)

            # ---------- attention for one pair, with filler interleave ----
            # Deferred softmax-normalization stages. The reciprocal/broadcast
            # chain of token span s is emitted one span LATER (between that
            # span's score groups) so it never sits ahead of ready matmuls
            # in the tensor queue. pending_norm carries stages across q4 and
            # pair boundaries.
            pending_norm = []

            def attention_pair(pr, fill_plan):
                """fill_plan: {sequential group idx -> [callables]} of
                projection / out-proj units, emitted between a group's exps
                and its AV matmuls so the PE has independent work while the
                ACT engine runs the exps."""
                q_tile = qk_tiles[(pr, 0)]
                k_tile = qk_tiles[(pr, 1)]
                gidx = 0
                pending_avs = []
                for q4 in range(NQ):
                    q0 = q4 * 512
                    nkb = 4 * (q4 + 1)
                    G = nkb // 2
                    avs = []
                    for hh in (0, 1):
                        av = av_ps.tile([HD + 1, 512], F32,
                                        name=f"av_{pr}_{hh}_{q4}", tag="av")
                        avs.append(av)
                    for g in range(G):
                        kbs = (2 * g, 2 * g + 1)
                        # kb-major tiles: each holds BOTH heads for one kb
                        # ([h0 | h1] across 2 banks) so exp fires after just
                        # the two concurrent score matmuls of that kb
                        sts = []
                        exs = []
                        for i, kb in enumerate(kbs):
                            s0 = max(0, kb * P - q0)
                            st = st_ps.tile([P, 1024], F32,
                                            name=f"st_{pr}_{q4}_{g}_{i}",
                                            tag="st")
                            # h0/h1 back-to-back: K=64 pairs run in
                            # different PE row groups concurrently
                            pair_mms = []
                            for hh in (0, 1):
                                rows = slice(hh * HD, hh * HD + HD)
                                pair_mms.append(nc.tensor.matmul(
                                    st[:, hh * 512 + s0:hh * 512 + 512],
                                    lhsT=k_tile[rows, kb * P:(kb + 1) * P],
                                    rhs=q_tile[rows, q0 + s0:q0 + 512],
                                    start=True, stop=True))
                            tile.add_dep_helper(
                                pair_mms[1].ins, pair_mms[0].ins, sync=False,
                                reason="keep score pair adjacent")
                            pin_after[0] = pair_mms[0]
                            sts.append(st)
                            # exp: one wide call unless the diagonal trim
                            # makes two narrow calls cheaper
                            ex = ex_pool.tile([P, 1024], BF16,
                                              name=f"ex_{pr}_{q4}_{g}_{i}",
                                              tag="ex")
                            if s0 <= 2 * P:
                                nc.scalar.activation(
                                    out=ex[:, s0:1024], in_=st[:, s0:1024],
                                    func=mybir.ActivationFunctionType.Exp,
                                    scale=scale)
                            else:
                                nc.scalar.activation(
                                    out=ex[:, s0:512], in_=st[:, s0:512],
                                    func=mybir.ActivationFunctionType.Exp,
                                    scale=scale)
                                nc.scalar.activation(
                                    out=ex[:, 512 + s0:1024],
                                    in_=st[:, 512 + s0:1024],
                                    func=mybir.ActivationFunctionType.Exp,
                                    scale=scale)
                            # causal mask on the diagonal block (gpsimd:
                            # off the busier vector queue)
                            d0 = kb * P - q0
                            if d0 >= 0:
                                for hh in (0, 1):
                                    c = hh * 512 + d0
                                    nc.gpsimd.tensor_mul(
                                        out=ex[:, c:c + P],
                                        in0=ex[:, c:c + P], in1=tri_sb)
                            exs.append(ex)
                        # lagged AVs (2 groups back): their exps finished
                        # long ago, so they issue with zero wait and never
                        # split this group's concurrent score pairs
                        if len(pending_avs) >= 2:
                            pending_avs.pop(0)()

                        def emit_avs(exs=exs, kbs=kbs, q0=q0, nkb=nkb,
                                     avs=avs, pr=pr):
                            for i, kb in enumerate(kbs):
                                s0 = max(0, kb * P - q0)
                                for hh in (0, 1):
                                    h = 2 * pr + hh
                                    nc.tensor.matmul(
                                        avs[hh][:, s0:512],
                                        lhsT=v_all[:, kb, h, :],
                                        rhs=exs[i][:, hh * 512 + s0:
                                                   hh * 512 + 512],
                                        start=(kb == 0),
                                        stop=(kb == nkb - 1))
                        pending_avs.append(emit_avs)
                        # deferred norms + filler cover the ACT latency
                        if g <= 1 and pending_norm:
                            pending_norm.pop(0)()
                        for fn in fill_plan.get(gidx, ()):
                            fn()
                        gidx += 1
                    # flush remaining AVs before the epilogue
                    while pending_avs:
                        pending_avs.pop(0)()

                    # ---- q4 epilogue: evacuate now, normalize deferred ----
                    dr = drow_pool.tile([P, 512], BF16, name=f"dr_{pr}_{q4}",
                                        tag="drow")
                    tmpt = tmp_pool.tile([P, 512], BF16, name=f"tp_{pr}_{q4}",
                                         tag="tmp")
                    den = den_pool.tile([8, P], BF16, name=f"den_{pr}_{q4}",
                                        tag="den")
                    recl = recl_pool.tile([P, 2, 512], BF16,
                                          name=f"recl_{pr}_{q4}", tag="recl")
                    # h0: raw attn to attn_sb, den row to dr
                    nc.vector.tensor_copy(
                        out=attn_sb[0:HD, pr, q0:q0 + 512], in_=avs[0][0:HD, :])
                    nc.vector.tensor_copy(out=dr[HD:HD + 1, :],
                                          in_=avs[0][HD:HD + 1, :])
                    # h1: raw attn + den row to tmp
                    nc.vector.tensor_copy(out=tmpt[0:HD + 1, :], in_=avs[1])
                    # fold den rows [1,512] -> [4,128] for a cheap reciprocal
                    nc.gpsimd.dma_start(out=den[0:4, :], in_=dr[HD:HD + 1, :])
                    nc.gpsimd.dma_start(out=den[4:8, :],
                                        in_=tmpt[HD:HD + 1, :])

                    def norm_a(den=den, recl=recl):
                        with nc.allow_low_precision(
                                reason="bf16 rounding of softmax denom"):
                            nc.vector.reciprocal(out=den, in_=den)
                        # unfold to partition 64 (legal matmul base partition)
                        nc.gpsimd.dma_start(out=recl[HD:HD + 1, 0, :],
                                            in_=den[0:4, :])
                        nc.gpsimd.dma_start(out=recl[HD:HD + 1, 1, :],
                                            in_=den[4:8, :])

                    def norm_b(pr=pr, q0=q0, recl=recl, tmpt=tmpt):
                        # broadcast 1/den across 64 partitions via K=1 matmul
                        bpst = st_ps.tile([P, 1024], F32,
                                          name=f"bps_{pr}_{q0}", tag="st")
                        for hh in (0, 1):
                            nc.tensor.matmul(
                                bpst[0:HD, hh * 512:hh * 512 + 512],
                                lhsT=ones_sb[HD:HD + 1, 0:HD],
                                rhs=recl[HD:HD + 1, hh, :],
                                start=True, stop=True)
                        sl = attn_sb[0:HD, pr, q0:q0 + 512]
                        nc.vector.tensor_mul(out=sl, in0=sl,
                                             in1=bpst[0:HD, 0:512])
                        nc.vector.tensor_mul(out=tmpt[0:HD, :],
                                             in0=tmpt[0:HD, :],
                                             in1=bpst[0:HD, 512:1024])
                        nc.gpsimd.dma_start(
                            out=attn_sb[HD:P, pr, q0:q0 + 512],
                            in_=tmpt[0:HD, :])

                    pending_norm.append(norm_a)
                    pending_norm.append(norm_b)

            # ---------- emission schedule ----------
            # Minimal prologue: just what pair-0 span 0 needs. The rest of
            # the V/QK0 projections ride inside pair 0's attention as
            # deadline-constrained filler — this keeps the attention region
            # PE-bound (ACT would otherwise dominate it and the PE clock
            # gate would oscillate).
            for kb in range(4):
                v_unit(kb)
            qk_unit(0, 0, 0)
            qk_unit(0, 1, 0)

            # Pairs: each pair's attention (20 groups, indices 0..19 —
            # q4=0: 0-1, q4=1: 2-5, q4=2: 6-11, q4=3: 12-19) carries filler
            # units, each tagged with the earliest legal group.
            def build_plan(jobs, nslots=20, heat_from=20):
                """jobs: [(min_slot, max_slot, callable)] -> {slot: [fns]},
                greedily balancing load within each job's legal window.
                Empty slots get a heater; slots >= heat_from get one extra
                (late-pair insurance against clock-gate collapse)."""
                plan = collections.defaultdict(list)
                for ms, mx, fn in sorted(jobs, key=lambda j: (j[1], j[0])):
                    slot = min(range(ms, mx + 1),
                               key=lambda s_: (len(plan[s_]), s_))
                    plan[slot].append(fn)
                return plan

            for pr in range(PAIRS):
                jobs = []
                if pr == 0:
                    # rest of the V projection and own QK, due before the
                    # span that consumes them (q4=s starts at slot
                    # {1:2, 2:6, 3:12})
                    win = {1: (0, 1), 2: (2, 5), 3: (6, 11)}
                    for blk in (1, 2, 3):
                        lo, hi = win[blk]
                        jobs += [(lo, hi, lambda kb=kb: v_unit(kb))
                                 for kb in range(4 * blk, 4 * blk + 4)]
                        jobs += [(lo, hi,
                                  lambda t=t, blk=blk: qk_unit(0, t, blk))
                                 for t in (0, 1)]
                if pr + 1 < PAIRS:
                    # next pair's qk units (pair 2 keeps mc3 back: pair 3
                    # fills its own first groups with them)
                    jobs += [
                        (0, 19, lambda t=t, mc=mc: qk_unit(pr + 1, t, mc))
                        for mc in range(MC if pr < 2 else MC - 1)
                        for t in (0, 1)]
                else:
                    jobs += [(0, 0, lambda: qk_unit(3, 0, 3)),
                             (1, 1, lambda: qk_unit(3, 1, 3))]
                if pr == 1:
                    # out-proj part A (pairs 0+1) for spans this pair has
                    # normalized: span s legal once norm_b(p1, s) popped
                    # (q4=s+1, g>=2)
                    span_min = {0: 4, 1: 8}
                    for s in (0, 1):
                        jobs += [
                            (span_min[s], 19,
                             lambda qt=qt, fc=fc: oproj_a(qt, fc))
                            for qt in range(4 * s, 4 * s + 4)
                            for fc in (0, 1)]
                elif pr == 2:
                    # part A for spans 2-3; span 3 must wait for pair 1's
                    # deferred span-3 norms (popped at slots 0-1)
                    jobs += [
                        (0 if qt < 12 else 2, 19,
                         lambda qt=qt, fc=fc: oproj_a(qt, fc))
                        for qt in range(8, KB) for fc in (0, 1)]
                elif pr == 3:
                    # part B (all pairs) for spans this pair has finished
                    span_min = {0: 4, 1: 8, 2: 14}
                    for s in (0, 1, 2):
                        jobs += [
                            (span_min[s], 19,
                             lambda qt=qt, fc=fc: oproj_b(qt, fc))
                            for qt in range(4 * s, 4 * s + 4)
                            for fc in (0, 1)]
                attention_pair(pr, build_plan(jobs))

            # flush deferred normalizations, then the last token span's
            # out-projection part B
            while pending_norm:
                pending_norm.pop(0)()
            for qt in range(12, KB):
                for fc in (0, 1):
                    oproj_b(qt, fc)

    nc.compile()
    return nc


def make_core_inputs(x, Wqkv_w, Wqkv_b, out_w, out_b, H, n_tp):
    """Host-side shard + layout prep. Returns list of in_maps (one per core).
    Core c handles batch c // n_tp, head group c % n_tp."""
    B, L, D = x.shape
    hpg = H // n_tp            # heads per core
    EV = hpg * HD
    ET = 2 * hpg * HD // P
    bf = ml_dtypes.bfloat16
    tri = np.triu(np.ones((P, P), dtype=np.float32))  # [k, q]: 1 if q >= k
    in_maps = []
    for c in range(B * n_tp):
        b, g = c // n_tp, c % n_tp
        # qk row order: per pair p -> q(2p), q(2p+1), k(2p), k(2p+1)
        qk_rows = []
        for p_ in range(hpg // 2):
            for h in (2 * p_, 2 * p_ + 1):
                qk_rows.extend(range(g * hpg * HD + h * HD,
                                     g * hpg * HD + h * HD + HD))
            for h in (2 * p_, 2 * p_ + 1):
                qk_rows.extend(range(D + g * hpg * HD + h * HD,
                                     D + g * hpg * HD + h * HD + HD))
        qk_rows = np.array(qk_rows)
        v_rows = np.arange(2 * D + g * EV, 2 * D + (g + 1) * EV)
        in_maps.append({
            "xT": np.ascontiguousarray(x[b].T).astype(bf),
            "wqkT": np.ascontiguousarray(Wqkv_w[qk_rows].T).astype(bf),
            "wvT": np.ascontiguousarray(Wqkv_w[v_rows].T).astype(bf),
            "bqk": np.ascontiguousarray(
                Wqkv_b[qk_rows].reshape(ET, P).T).astype(np.float32),
            "vb": np.tile(Wqkv_b[v_rows], (P, 1)).astype(bf),
            "woT": np.ascontiguousarray(
                out_w[:, g * EV:(g + 1) * EV].T).astype(bf),
            "ob": (np.tile(out_b, (P, 1)).astype(np.float32) if g == 0
                   else np.zeros((P, D), np.float32)),
            "tri": tri.astype(bf),
            "onesd": np.ones((P, 512), np.float32).astype(bf),
        })
    return in_maps


_NC_CACHE = {}
LAST_RESULTS = None


def kernel(x, Wqkv_w, Wqkv_b, out_w, out_b):
    global LAST_RESULTS
    x = np.asarray(x, dtype=np.float32)
    Wqkv_w = np.asarray(Wqkv_w, dtype=np.float32)
    Wqkv_b = np.asarray(Wqkv_b, dtype=np.float32)
    out_w = np.asarray(out_w, dtype=np.float32)
    out_b = np.asarray(out_b, dtype=np.float32)

    B, L, D = x.shape
    H = 16
    n_tp = 2
    hpg = H // n_tp

    key = (L, D, hpg)
    if key not in _NC_CACHE:
        _NC_CACHE[key] = build_mha_nc(L, D, hpg)
    nc = _NC_CACHE[key]

    in_maps = make_core_inputs(x, Wqkv_w, Wqkv_b, out_w, out_b, H, n_tp)

    from concourse.bass_utils import run_bass_kernel_spmd
    res = run_bass_kernel_spmd(nc, in_maps, core_ids=list(range(len(in_maps))))
    LAST_RESULTS = res

    out = np.empty((B, L, D), dtype=np.float32)
    for b in range(B):
        out[b] = res.results[n_tp * b]["out"]
        for g in range(1, n_tp):
            out[b] += res.results[n_tp * b + g]["out"]
    return out


if __name__ == "__main__":
    nc = build_mha_nc(2048, 1024, 8)
    print("built OK")
